# revision 23
# baseline (speedup 1.0000x reference)
"""GNN spiral-conv encoder on 8 TRN2 NeuronCores (Bass/Tile).

Sharding: data-parallel over batch (2 of 16 samples per core); all index
structures replicated. Final Linear uses bf16 Wf replicated per core.

Per level: per-slot f32 x tables in HBM; dma_gather pulls, per used
destination vertex and spiral slot, the source row into [128 dest, elem]
slabs; PE transpose-matmuls assemble the conv rhs in PSUM; matmuls vs
host-built block-diagonal weight chunks accumulate; bias+ELU
(= Relu(z+b) + exp(min(z+b,0)) - 1) splits across DVE and the otherwise
idle ACT engine. h rows are transposed back (PE) into f32 HBM row tables.
(A transpose=True dma_gather path that would skip the PE assembly is
gated off: it crashes this HW/ucode build; see the new_conv flag.)

Level 0's h table packs two h rows per 512B table row (chunk-half pairing:
dest j and j+64 of each 128-chunk share a row) so pool-0 indices fit int16
in one pass; the pool then runs two masked matmuls per slab (half A/B
banded value matrices).

Pools: entries sorted by dest; window gathers are issued one ahead
(software pipeline, bufs=3) instead of on demand; PE matmul vs banded
value matrix accumulates x_{i+1}^T in PSUM; levels 1-3 keep the legacy
f32 tables and (for 2/3) the legacy assembly conv. Level-3 pool stores
the x4^T planes directly in the final matmul's layout (no transpose).
"""
import sys

sys.path.insert(0, "/opt/trn_rl_repo")

import numpy as np

import concourse.bass as bass
import concourse.tile as tile
from concourse import bacc, mybir
from concourse.bass_utils import run_bass_kernel_spmd
from concourse.library_config import mlp as _mlp_lib
from concourse.masks import make_identity

F32 = mybir.dt.float32
BF16 = mybir.dt.bfloat16
I16 = mybir.dt.int16
AF = mybir.ActivationFunctionType

VERTS = [65536, 16384, 4096, 1024, 256]
SEQ = 9
CH = [3, 32, 64, 128, 256]
LATENT = 256
B = 16
N_CORES = 8
B_LOC = B // N_CORES

SGC = [2048, 2048, 1024, 512]   # spiral gather idxs per instruction
PGC = [1024, 1024, 1024, 256]    # pool gather idxs per instruction
RWIN = 64                        # pool value-matrix r-window
RBLOCK = 512                     # pool PSUM r-block
HALF = 32768


def _wrap_idx16(idx, chunk):
    """[128, n/16] int16 dma_gather layout: within each `chunk` window,
    index i -> partition i%16, col i//16; replicated to all 8 groups."""
    idx = np.asarray(idx, dtype=np.int64)
    n = idx.shape[0]
    assert n % chunk == 0 and idx.max() < HALF and idx.min() >= 0
    nin = n // chunk
    w = idx.reshape(nin, chunk // 16, 16).astype(np.int16)
    blocks = [w[j].T for j in range(nin)]
    one = np.concatenate(blocks, axis=1)  # [16, n/16]
    return np.tile(one, (8, 1))           # [128, n/16]


def _pad_to(a, n, fill=0):
    out = np.full((n,) + a.shape[1:], fill, dtype=a.dtype)
    out[: a.shape[0]] = a
    return out


def _h0_row_half(hrow):
    """Level-0 paired h-table position for h row index (1-based; 0 = zero
    row). Dest pos p = hrow-1; chunk base cb = (p//128)*128; j = p%128;
    row = cb//2 + (j%64) + 1, half = j//64."""
    if hrow == 0:
        return 0, 0
    p = hrow - 1
    cb, j = divmod(p, 128)
    return cb * 64 + (j % 64) + 1, j // 64


def _build_level_host(i, idx, col, row, val, W, b):
    N_in, N_out = VERTS[i], VERTS[i + 1]
    C_in, C_out = CH[i], CH[i + 1]
    L = dict(N_in=N_in, N_out=N_out, C_in=C_in, C_out=C_out)
    L["xe"] = max(64, 2 * C_in)      # legacy x-table row width (f32)
    L["he"] = max(64, 2 * C_out)     # h-table row width (f32)
    M = 2 * C_out
    L["M"] = M
    L["n_mh"] = -(-M // 128)
    L["new_conv"] = False  # transpose-gather conv pending HW validation

    # used conv outputs (h rows) = unique pool source columns
    used = np.unique(col)
    U = used.shape[0]
    Upad = -(-U // SGC[i]) * SGC[i]
    L.update(used=used, U=U, Upad=Upad)

    # spiral gather index lists per slot s
    src = idx[used, :]  # [U, SEQ]
    gcols = []
    ref_lists = []
    for s in range(SEQ):
        ss = src[:, s]
        if i == 0:
            ref = np.unique(ss)
            assert ref.shape[0] < HALF, ref.shape
            loc = np.searchsorted(ref, ss) + 1
            ref_lists.append(ref)
        else:
            loc = ss + 1
        gcols.append(_wrap_idx16(_pad_to(loc.astype(np.int64), Upad), SGC[i]))
    L["gidx"] = np.concatenate(gcols, axis=1)  # [128, SEQ*Upad/16]
    L["ref_lists"] = ref_lists

    if L["new_conv"]:
        # per-slot block-diagonal weight chunks [Ks, M] (slot 0 gets a
        # bias row fed by the ones-component of its x table)
        Wbs = []
        for s in range(SEQ):
            Ks = 2 * C_in + (1 if s == 0 else 0)
            Wc = np.zeros((Ks, M), dtype=np.float32)
            for bb in range(B_LOC):
                Wc[bb * C_in:(bb + 1) * C_in,
                   bb * C_out:(bb + 1) * C_out] = W[s * C_in:(s + 1) * C_in, :]
                if s == 0:
                    Wc[2 * C_in, bb * C_out:(bb + 1) * C_out] = b
            Wbs.append(Wc)
        L["Wbs"] = Wbs
    else:
        # legacy conv K-chunks; level 0 instead packs all 9 slots into
        # one gapless 54-row chunk (strided DVE pack-copy + one PE
        # transpose per 128-dest sub-block)
        if i == 0:
            kw = 2 * C_in
            Wc = np.zeros((SEQ * kw, M), dtype=np.float32)
            for s in range(SEQ):
                for bb in range(B_LOC):
                    Wc[s * kw + bb * C_in: s * kw + (bb + 1) * C_in,
                       bb * C_out:(bb + 1) * C_out] = \
                        W[s * C_in:(s + 1) * C_in, :]
            L.update(kchunks=None, Wcs=[Wc])
        else:
            if 2 * C_in <= 128:
                kwidth = 2 * C_in
                raw = [(s, None) for s in range(SEQ)]
            else:
                kwidth = C_in
                raw = [(s, bb) for s in range(SEQ) for bb in range(B_LOC)]
            if kwidth <= 32:
                offs = [0, 32, 64]
            elif kwidth <= 64:
                offs = [0, 64]
            else:
                offs = [0]
            cs = len(offs)
            kchunks = []
            for j in range(0, len(raw), cs):
                kchunks.append([(s, bsel, kwidth, offs[t])
                                for t, (s, bsel) in enumerate(raw[j: j + cs])])
            Wcs = []
            for ck in kchunks:
                K = max(poff + kwidth for (_, _, _, poff) in ck)
                Wc = np.zeros((K, M), dtype=np.float32)
                for (s, bsel, kw, poff) in ck:
                    if bsel is None:
                        for bb in range(B_LOC):
                            Wc[poff + bb * C_in: poff + (bb + 1) * C_in,
                               bb * C_out:(bb + 1) * C_out] = \
                                W[s * C_in:(s + 1) * C_in, :]
                    else:
                        Wc[poff: poff + C_in,
                           bsel * C_out:(bsel + 1) * C_out] = \
                            W[s * C_in:(s + 1) * C_in, :]
                Wcs.append(Wc)
            L.update(kchunks=kchunks, Wcs=Wcs)
        bias_flat = np.repeat(b[None, :], B_LOC, axis=0).reshape(-1)  # [M]
        L["bias"] = _pad_to(bias_flat.astype(np.float32), L["n_mh"] * 128) \
            .reshape(L["n_mh"], 128).T.copy()  # [128, n_mh]

    # pool: sort entries by dest row, pack into slabs
    colpos = np.searchsorted(used, col)  # h row index - 1
    order = np.argsort(row, kind="stable")
    er, ec, ev = row[order], colpos[order] + 1, val[order]
    slabs, cur = [], []
    for k in range(er.shape[0]):
        r = int(er[k])
        if cur and (len(cur) >= 128 or r - cur[0][2] >= RWIN
                    or (r // RBLOCK) != (cur[0][2] // RBLOCK)):
            slabs.append(cur)
            cur = []
        cur.append((int(ec[k]), float(ev[k]), r))
    if cur:
        slabs.append(cur)
    nslab = len(slabs)
    paired = i == 0
    n_half = 2 if paired else 1
    pool_idx = np.zeros((nslab * 128,), dtype=np.int64)
    S2 = np.zeros((128, nslab * n_half * RWIN), dtype=np.float32)
    slab_meta = []
    for si, sl in enumerate(slabs):
        r0 = sl[0][2]
        g = r0 // RBLOCK
        w_off = r0 - g * RBLOCK
        if w_off + RWIN > RBLOCK:
            w_off = RBLOCK - RWIN
        slab_meta.append((g, w_off))
        for j, (hrow, v, r) in enumerate(sl):
            wpos = r - g * RBLOCK - w_off
            if paired:
                trow, half = _h0_row_half(hrow)
                pool_idx[si * 128 + j] = trow
                S2[j, (2 * si + half) * RWIN + wpos] = v
            else:
                pool_idx[si * 128 + j] = hrow
                S2[j, si * RWIN + wpos] = v
    rblocks = []
    cur_g, start = None, 0
    for si, (g, _) in enumerate(slab_meta):
        if g != cur_g:
            if cur_g is not None:
                rblocks.append((cur_g, start, si))
            cur_g, start = g, si
    if cur_g is not None:
        rblocks.append((cur_g, start, nslab))
    covered = {g for (g, _, _) in rblocks}
    for g in range(-(-N_out // RBLOCK)):
        if g not in covered:
            rblocks.append((g, 0, 0))
    rblocks.sort()
    npad = -(-nslab * 128 // PGC[i]) * PGC[i]
    L["pidx"] = _wrap_idx16(_pad_to(pool_idx, npad), PGC[i])
    L.update(nslab=nslab, paired=paired, S2=S2, slab_meta=slab_meta,
             rblocks=rblocks, npad=npad)
    return L


def _host_prep(inputs):
    g = lambda k: np.asarray(inputs[k])
    return [
        _build_level_host(i, g(f"idx{i}").astype(np.int64),
                          g(f"col{i}").astype(np.int64),
                          g(f"row{i}").astype(np.int64),
                          g(f"val{i}").astype(np.float32),
                          g(f"W{i}").astype(np.float32),
                          g(f"b{i}").astype(np.float32))
        for i in range(4)
    ]


def _build_in_map(levels, inputs, core):
    import ml_dtypes
    x = np.asarray(inputs["x"], dtype=np.float32)
    xs = x[core * B_LOC:(core + 1) * B_LOC]  # [2, N0, 3]
    m = {}
    L0 = levels[0]
    for s in range(SEQ):
        ref = L0["ref_lists"][s]
        if L0["new_conv"]:
            t = np.zeros((HALF, 128), dtype=ml_dtypes.bfloat16)
            for bb in range(B_LOC):
                t[1:1 + ref.shape[0], bb * CH[0]:(bb + 1) * CH[0]] = xs[bb][ref, :]
            if s == 0:
                t[1:1 + ref.shape[0], 2 * CH[0]] = 1.0  # bias ones-component
        else:
            t = np.zeros((HALF, 64), dtype=np.float32)
            for bb in range(B_LOC):
                t[1:1 + ref.shape[0], bb * CH[0]:(bb + 1) * CH[0]] = xs[bb][ref, :]
        m[f"x0t{s}"] = t
    for i, L in enumerate(levels):
        m[f"gidx{i}"] = L["gidx"]
        m[f"pidx{i}"] = L["pidx"]
        m[f"S2_{i}"] = L["S2"]
        if L["new_conv"]:
            for s, Wc in enumerate(L["Wbs"]):
                m[f"Wb{i}_{s}"] = Wc
        else:
            for k, Wc in enumerate(L["Wcs"]):
                m[f"W{i}_{k}"] = Wc
            m[f"bias{i}"] = L["bias"]
    m["Wfb"] = np.asarray(inputs["Wf"], dtype=np.float32).astype(ml_dtypes.bfloat16)
    m["bfv"] = np.asarray(inputs["bf"], dtype=np.float32)[:, None]
    return m


def _build_bass(levels):
    nc = bacc.Bacc("TRN2", target_bir_lowering=False, debug=False,
                   num_devices=N_CORES, num_swdge_queues=4)
    d = {}
    x0w, x0dt = ((128, BF16) if levels[0]["new_conv"] else (64, F32))
    for s in range(SEQ):
        d[f"x0t{s}"] = nc.dram_tensor(f"x0t{s}", [HALF, x0w], x0dt,
                                      kind="ExternalInput")
    for i, L in enumerate(levels):
        d[f"gidx{i}"] = nc.dram_tensor(
            f"gidx{i}", [128, SEQ * L["Upad"] // 16], I16, kind="ExternalInput")
        d[f"pidx{i}"] = nc.dram_tensor(
            f"pidx{i}", [128, L["npad"] // 16], I16, kind="ExternalInput")
        n_half = 2 if L["paired"] else 1
        d[f"S2_{i}"] = nc.dram_tensor(
            f"S2_{i}", [128, L["nslab"] * n_half * RWIN], F32,
            kind="ExternalInput")
        if L["new_conv"]:
            for s, Wc in enumerate(L["Wbs"]):
                d[f"Wb{i}_{s}"] = nc.dram_tensor(
                    f"Wb{i}_{s}", list(Wc.shape), F32, kind="ExternalInput")
        else:
            for k, Wc in enumerate(L["Wcs"]):
                d[f"W{i}_{k}"] = nc.dram_tensor(f"W{i}_{k}", list(Wc.shape),
                                                F32, kind="ExternalInput")
            d[f"bias{i}"] = nc.dram_tensor(f"bias{i}", [128, L["n_mh"]], F32,
                                           kind="ExternalInput")
        if i == 1:
            if L["new_conv"]:
                # bf16 x table, ones-component at col 2*C_in for the bias row
                d["xt1"] = nc.dram_tensor("xt1", [L["N_in"] + 1, 128], BF16,
                                          kind="Internal")
            else:
                d["xt1"] = nc.dram_tensor("xt1", [L["N_in"] + 1, L["xe"]],
                                          F32, kind="Internal")
        elif i > 1:
            d[f"xt{i}"] = nc.dram_tensor(f"xt{i}", [L["N_in"] + 1, L["xe"]],
                                         F32, kind="Internal")
        if i == 0:
            d["hta0"] = nc.dram_tensor("hta0", [L["Upad"] // 2 + 1, 128], F32,
                                       kind="Internal")
        else:
            d[f"hta{i}"] = nc.dram_tensor(f"hta{i}", [L["Upad"] + 1, L["he"]],
                                          F32, kind="Internal")
    d["x4p"] = nc.dram_tensor("x4p", [4, 128, 256], F32, kind="Internal")
    d["Wfb"] = nc.dram_tensor("Wfb", [VERTS[4] * CH[4], LATENT], BF16,
                              kind="ExternalInput")
    d["bfv"] = nc.dram_tensor("bfv", [LATENT, 1], F32, kind="ExternalInput")
    d["out"] = nc.dram_tensor("out", [B_LOC, LATENT], F32, kind="ExternalOutput")

    with tile.TileContext(nc) as tc:
        nc.gpsimd.load_library(_mlp_lib)
        _emit(nc, tc, d, levels)
    nc.compile()
    return nc


def _emit(nc, tc, d, levels):
    from contextlib import ExitStack
    import itertools
    with ExitStack() as ctx:
        p = {}
        # SWDGE sem lanes are assigned round-robin over Pool-engine DMA
        # instructions in emission order (8 lanes); queue_num must follow
        # emission order so each lane stays locked to one queue.
        p["qctr"] = itertools.count()
        p["ident"] = ctx.enter_context(tc.tile_pool(name="ident", bufs=1))
        p["w"] = ctx.enter_context(tc.tile_pool(name="wp", bufs=1))
        p["idx"] = ctx.enter_context(tc.tile_pool(name="idxp", bufs=3))
        p["g"] = ctx.enter_context(tc.tile_pool(name="gp", bufs=2))
        p["rhs"] = ctx.enter_context(tc.tile_pool(name="rhsp", bufs=3))
        p["h"] = ctx.enter_context(tc.tile_pool(name="hp", bufs=3))
        p["tmp"] = ctx.enter_context(tc.tile_pool(name="tmpp", bufs=2))
        p["nat"] = ctx.enter_context(tc.tile_pool(name="natp", bufs=2))
        p["s2"] = ctx.enter_context(tc.tile_pool(name="s2p", bufs=2))
        p["ph"] = ctx.enter_context(tc.tile_pool(name="php", bufs=3))
        p["fin"] = ctx.enter_context(tc.tile_pool(name="finp", bufs=1))
        p["wfq"] = ctx.enter_context(tc.tile_pool(name="wfqp", bufs=3))
        p["asm_ps"] = ctx.enter_context(
            tc.tile_pool(name="asmps", bufs=1, space="PSUM"))
        p["conv_ps"] = ctx.enter_context(
            tc.tile_pool(name="convps", bufs=1, space="PSUM"))
        p["nat_ps"] = ctx.enter_context(
            tc.tile_pool(name="natps", bufs=2, space="PSUM"))
        p["pool_ps"] = ctx.enter_context(
            tc.tile_pool(name="poolps", bufs=1, space="PSUM"))

        ident = p["ident"].tile([128, 128], F32)
        make_identity(nc, ident[:])
        zrow = p["ident"].tile([1, 512], F32)
        nc.vector.memset(zrow[:], 0.0)
        zrowb = p["ident"].tile([1, 128], BF16)
        nc.vector.memset(zrowb[:], 0.0)
        if levels[1]["new_conv"]:
            nc.sync.dma_start(d["xt1"][0:1, :], zrowb[:1, :])
        else:
            nc.sync.dma_start(d["xt1"][0:1, :], zrow[:1, :levels[1]["xe"]])
        for i in range(2, 4):
            nc.sync.dma_start(d[f"xt{i}"][0:1, :], zrow[:1, :levels[i]["xe"]])
        nc.sync.dma_start(d["hta0"][0:1, :], zrow[:1, :128])
        for i in range(1, 4):
            nc.sync.dma_start(d[f"hta{i}"][0:1, :], zrow[:1, :levels[i]["he"]])

        for i, L in enumerate(levels):
            if L["new_conv"]:
                _emit_conv_new(nc, d, i, L, ident, p)
            else:
                _emit_conv_legacy(nc, d, i, L, ident, p)
            _emit_pool(nc, d, levels, i, L, ident, p)
        _emit_final(nc, d, p)


def _emit_conv_new(nc, d, i, L, ident, p):
    """Levels 0/1: transpose-gather bf16 conv (no PE assembly)."""
    C_in, M = L["C_in"], L["M"]
    Upad, sgc = L["Upad"], SGC[i]
    n_gi = Upad // sgc

    Wb = []
    for s in range(SEQ):
        Ks = L["Wbs"][s].shape[0]
        wt32 = p["w"].tile([Ks, M], F32, tag=f"Wb32_{i}_{s}")
        nc.sync.dma_start(wt32[:], d[f"Wb{i}_{s}"][:])
        wtb = p["w"].tile([Ks, M], BF16, tag=f"Wb_{i}_{s}")
        nc.vector.tensor_copy(wtb[:], wt32[:])
        Wb.append(wtb)

    for gi in range(n_gi):
        gts = []
        for s in range(SEQ):
            it = p["idx"].tile([128, sgc // 16], I16, tag="gidx")
            nc.sync.dma_start(
                it[:], d[f"gidx{i}"][:, (s * Upad + gi * sgc) // 16:
                                     (s * Upad + (gi + 1) * sgc) // 16])
            gt = p["g"].tile([128, sgc], BF16, tag=f"g{s}")
            src_ap = d[f"x0t{s}"][:] if i == 0 else d["xt1"][:]
            nc.gpsimd.dma_gather(
                gt[:].rearrange("p (o n) -> p o n", o=1),
                src_ap, it[:], sgc, sgc, 128,
                transpose=True, queue_num=next(p["qctr"]) % 4)
            gts.append(gt)
        for blk in range(sgc // 512):
            if blk % 2 == 0:
                cps = p["conv_ps"].tile([128, 512], F32, tag="conv")
            else:
                cps = p["asm_ps"].tile([128, 512], F32, tag="asm")
            for s in range(SEQ):
                Ks = L["Wbs"][s].shape[0]
                nc.tensor.matmul(
                    cps[:M, :],
                    lhsT=Wb[s][:Ks, :M],
                    rhs=gts[s][:Ks, blk * 512:(blk + 1) * 512],
                    start=(s == 0), stop=(s == SEQ - 1))
            # ELU: h = Relu(z) + exp(min(z, 0)) - 1
            tneg = p["tmp"].tile([128, 512], F32, tag="tneg")
            nc.vector.tensor_scalar_min(tneg[:M, :], cps[:M, :], 0.0)
            nc.scalar.activation(tneg[:M, :], tneg[:M, :], AF.Exp)
            hT = p["h"].tile([128, 512], F32, tag="hT")
            nc.scalar.activation(hT[:M, :], cps[:M, :], AF.Relu)
            nc.vector.tensor_add(hT[:M, :], hT[:M, :], tneg[:M, :])
            nc.vector.tensor_scalar_add(hT[:M, :], hT[:M, :], -1.0)
            # transpose 128-dest chunks and store h rows
            for cc in range(4):
                nps = p["nat_ps"].tile([128, 128], F32, tag="tp")
                nc.tensor.matmul(nps[:, :M],
                                 lhsT=hT[:M, cc * 128:(cc + 1) * 128],
                                 rhs=ident[:M, :M], is_transpose=True,
                                 start=True, stop=True)
                nsb = p["nat"].tile([128, 128], F32, tag="hnatsb")
                nc.vector.tensor_copy(nsb[:, :M], nps[:, :M])
                pos0 = gi * sgc + blk * 512 + cc * 128
                if i == 0:
                    t0 = pos0 // 2 + 1
                    nc.sync.dma_start(d["hta0"][t0: t0 + 64, 0:64],
                                      nsb[0:64, :64])
                    nc.sync.dma_start(d["hta0"][t0: t0 + 64, 64:128],
                                      nsb[64:128, :64])
                else:
                    nc.sync.dma_start(
                        d[f"hta{i}"][pos0 + 1: pos0 + 129, :M], nsb[:, :M])


def _emit_conv_legacy(nc, d, i, L, ident, p):
    C_in, C_out, M, n_mh = L["C_in"], L["C_out"], L["M"], L["n_mh"]
    xe = L["xe"]
    Upad = L["Upad"]
    sgc = SGC[i]
    kchunks = L["kchunks"]

    Wts = []
    for k, Wc in enumerate(L["Wcs"]):
        wt = p["w"].tile([Wc.shape[0], Wc.shape[1]], F32, tag=f"W{i}_{k}")
        nc.sync.dma_start(wt[:], d[f"W{i}_{k}"][:])
        Wts.append(wt)
    bias_t = p["w"].tile([128, n_mh], F32, tag=f"bias{i}")
    nc.sync.dma_start(bias_t[:], d[f"bias{i}"][:])

    n_gi = Upad // sgc
    nslab_g = sgc // 128
    packed = i == 0  # gapless 54-row chunk via DVE pack + one transpose
    sw = nslab_g * xe  # per-slot region width (1024 at every level)

    for gi in range(n_gi):
        gbig = p["g"].tile([128, SEQ * sw], F32, tag="gbig")
        gtiles = [gbig[:, s * sw:(s + 1) * sw] for s in range(SEQ)]
        for s in range(SEQ):
            it = p["idx"].tile([128, sgc // 16], I16, tag="gidx")
            nc.sync.dma_start(
                it[:], d[f"gidx{i}"][:, (s * Upad + gi * sgc) // 16:
                                     (s * Upad + (gi + 1) * sgc) // 16])
            src_ap = d[f"x0t{s}"][:] if i == 0 else d[f"xt{i}"][:]
            nc.gpsimd.dma_gather(
                gtiles[s].rearrange("p (n e) -> p n e", e=xe),
                src_ap, it[:], sgc, sgc, xe,
                single_packet=False, queue_num=next(p["qctr"]) % 4)
        if packed:
            kw2 = 2 * C_in           # 6
            Kp = SEQ * kw2           # 54
            g4 = gbig[:].rearrange("p (s w e) -> p s w e",
                                   s=SEQ, w=nslab_g)
            for blk in range(nslab_g // 4):
                cps = p["conv_ps"].tile([128, 512 * n_mh], F32, tag="conv")
                aps = p["asm_ps"].tile([128, 512], F32, tag="asm")
                for sub in range(4):
                    slab_i = blk * 4 + sub
                    pk = p["rhs"].tile([128, 64], F32, tag="pk")
                    nc.vector.tensor_copy(
                        pk[:, :Kp].rearrange("p (s o e) -> p s o e",
                                             s=SEQ, o=1),
                        g4[:, :, slab_i: slab_i + 1, 0: kw2])
                    nc.tensor.matmul(
                        aps[0: Kp, sub * 128:(sub + 1) * 128],
                        lhsT=pk[:, :Kp], rhs=ident[:],
                        is_transpose=True, start=True, stop=True)
                rb = p["rhs"].tile([128, 512], F32, tag="rhs")
                nc.vector.tensor_copy(rb[:Kp, :], aps[:Kp, :])
                nc.tensor.matmul(
                    cps[:M, :512], lhsT=Wts[0][:, :M],
                    rhs=rb[:Kp, :], start=True, stop=True)
                _emit_elu_store(nc, d, i, L, ident, p, bias_t, cps, gi, blk)
            continue
        for blk in range(nslab_g // 4):
            cps = p["conv_ps"].tile([128, 512 * n_mh], F32, tag="conv")
            for kci, ck in enumerate(kchunks):
                K = max(poff + kw for (_, _, kw, poff) in ck)
                aps = p["asm_ps"].tile([128, 512], F32, tag="asm")
                for sub in range(4):
                    slab_i = blk * 4 + sub
                    for (s, bsel, kw, poff) in ck:
                        c0 = slab_i * xe + (0 if bsel is None
                                            else bsel * C_in)
                        in_ap = gtiles[s][:, c0: c0 + kw]
                        nc.tensor.matmul(
                            aps[poff: poff + kw,
                                sub * 128:(sub + 1) * 128],
                            lhsT=in_ap, rhs=ident[:],
                            start=True, stop=True)
                rb = p["rhs"].tile([128, 512], F32, tag="rhs")
                nc.vector.tensor_copy(rb[:K, :], aps[:K, :])
                for mh in range(n_mh):
                    mw = min(128, M - mh * 128)
                    nc.tensor.matmul(
                        cps[:mw, mh * 512:(mh + 1) * 512],
                        lhsT=Wts[kci][:, mh * 128: mh * 128 + mw],
                        rhs=rb[:K, :], start=(kci == 0),
                        stop=(kci == len(kchunks) - 1))
            _emit_elu_store(nc, d, i, L, ident, p, bias_t, cps, gi, blk)


def _emit_elu_store(nc, d, i, L, ident, p, bias_t, cps, gi, blk):
    M, n_mh = L["M"], L["n_mh"]
    sgc = SGC[i]
    for mh in range(n_mh):
        mw = min(128, M - mh * 128)
        # bias + ELU: h = Relu(z+b) + exp(min(z+b, 0)) - 1
        hT = p["h"].tile([128, 512], F32, tag="hT")
        tneg = p["tmp"].tile([128, 512], F32, tag="tneg")
        bsl = bias_t[:mw, mh: mh + 1]
        csl = cps[:mw, mh * 512:(mh + 1) * 512]
        nc.vector.tensor_scalar(tneg[:mw, :], csl, bsl, 0.0,
                                mybir.AluOpType.add,
                                mybir.AluOpType.min)
        nc.scalar.activation(tneg[:mw, :], tneg[:mw, :], AF.Exp)
        nc.scalar.activation(hT[:mw, :], csl, AF.Relu, bias=bsl)
        nc.vector.tensor_add(hT[:mw, :], hT[:mw, :], tneg[:mw, :])
        nc.vector.tensor_scalar_add(hT[:mw, :], hT[:mw, :], -1.0)
        # naturalize 128-dest chunks and store h rows
        for cc in range(4):
            nps = p["nat_ps"].tile([128, 128], F32, tag="tp")
            nc.tensor.matmul(nps[:, :mw],
                             lhsT=hT[:mw, cc * 128:(cc + 1) * 128],
                             rhs=ident[:mw, :mw], is_transpose=True,
                             start=True, stop=True)
            nsb = p["nat"].tile([128, 128], F32, tag="hnatsb")
            nc.vector.tensor_copy(nsb[:, :mw], nps[:, :mw])
            pos0 = gi * sgc + blk * 512 + cc * 128
            if i == 0:
                t0 = pos0 // 2 + 1
                nc.sync.dma_start(d["hta0"][t0: t0 + 64, 0:64],
                                  nsb[0:64, :64])
                nc.sync.dma_start(d["hta0"][t0: t0 + 64, 64:128],
                                  nsb[64:128, :64])
            else:
                nc.sync.dma_start(
                    d[f"hta{i}"][pos0 + 1: pos0 + 129,
                                 mh * 128: mh * 128 + mw],
                    nsb[:, :mw])


def _emit_pool(nc, d, levels, i, L, ident, p):
    nslab, npad = L["nslab"], L["npad"]
    paired = L["paired"]
    n_half = 2 if paired else 1
    pgc = PGC[i]
    helem = 128 if paired else L["he"]
    M_next = 512 if i == 3 else (2 * levels[i + 1]["C_in"])
    n_mh_next = -(-M_next // 128)
    win_slabs = pgc // 128
    n_win = npad // pgc
    tiles = {}

    def issue(wi):
        it = p["idx"].tile([128, pgc // 16], I16, tag="pidx")
        nc.sync.dma_start(
            it[:], d[f"pidx{i}"][:, wi * pgc // 16: (wi + 1) * pgc // 16])
        gt = p["ph"].tile([128, win_slabs * helem], F32, tag="ph")
        nc.gpsimd.dma_gather(
            gt[:].rearrange("p (n e) -> p n e", e=helem),
            d[f"hta{i}"][:], it[:], pgc, pgc, helem,
            single_packet=False, queue_num=next(p["qctr"]) % 4)
        tiles[wi] = gt

    for wi in range(min(2, n_win)):
        issue(wi)

    xt1_bf = i == 0 and levels[1]["new_conv"]
    if xt1_bf:
        xsb_a = p["nat"].tile([128, 128], BF16, tag="xsb0")
        xsb_b = p["nat"].tile([128, 128], BF16, tag="xsb0")
        xsb = [xsb_a, xsb_b]
        for t in xsb:
            nc.vector.memset(t[:], 0.0)
            nc.vector.memset(t[:, 64:65], 1.0)

    for (g, s0, s1) in L["rblocks"]:
        n_rc = min(RBLOCK, L["N_out"] - g * RBLOCK)
        s2t = None
        if s1 > s0:
            s2t = p["s2"].tile([128, (s1 - s0) * n_half * RWIN], F32, tag="s2")
            nc.sync.dma_start(
                s2t[:], d[f"S2_{i}"][:, s0 * n_half * RWIN:
                                     s1 * n_half * RWIN])
        for mh in range(n_mh_next):
            mw = min(128, M_next - mh * 128)
            pps = p["pool_ps"].tile([128, RBLOCK], F32, tag="pool")
            nc.vector.memset(pps[:mw, :], 0.0)
            for si in range(s0, s1):
                wi, sub = divmod(si, win_slabs)
                if wi + 1 < n_win and wi + 1 not in tiles:
                    issue(wi + 1)
                (_, w_off) = L["slab_meta"][si]
                gt = tiles[wi]
                for hf in range(n_half):
                    nc.tensor.matmul(
                        pps[:mw, w_off: w_off + RWIN],
                        lhsT=gt[:, sub * helem + hf * 64 + mh * 128:
                                sub * helem + hf * 64 + mh * 128 + mw]
                        if paired else
                        gt[:, sub * helem + mh * 128:
                           sub * helem + mh * 128 + mw],
                        rhs=s2t[:, ((si - s0) * n_half + hf) * RWIN:
                                ((si - s0) * n_half + hf + 1) * RWIN],
                        start=False,
                        stop=(si == s1 - 1 and hf == n_half - 1),
                        skip_group_check=True)
            xTs = p["nat"].tile([128, RBLOCK], F32, tag="xT")
            nc.vector.tensor_copy(xTs[:mw, :], pps[:mw, :])
            if i == 3:
                # store x4^T plane directly in final-matmul layout
                # plane index p = 2h + b for mh = 2b + h
                b_, h_ = divmod(mh, 2)
                nc.sync.dma_start(d["x4p"][2 * h_ + b_],
                                  xTs[:mw, :256])
                continue
            for cc in range(-(-n_rc // 128)):
                ncc = min(128, n_rc - cc * 128)
                nps = p["nat_ps"].tile([128, 128], F32, tag="tp")
                nc.tensor.matmul(nps[:ncc, :mw],
                                 lhsT=xTs[:mw, cc * 128: cc * 128 + ncc],
                                 rhs=ident[:mw, :mw], is_transpose=True,
                                 start=True, stop=True)
                row0 = g * RBLOCK + cc * 128 + 1
                if xt1_bf:
                    nsb = xsb[cc % 2]
                    nc.vector.tensor_copy(nsb[:ncc, :64], nps[:ncc, :64])
                    nc.sync.dma_start(
                        d["xt1"][row0: row0 + ncc, :], nsb[:ncc, :])
                else:
                    nsb = p["nat"].tile([128, 128], F32, tag="xnatsb")
                    nc.vector.tensor_copy(nsb[:ncc, :mw], nps[:ncc, :mw])
                    nc.sync.dma_start(
                        d[f"xt{i + 1}"][row0: row0 + ncc,
                                        mh * 128: mh * 128 + mw],
                        nsb[:ncc, :mw])


def _emit_final(nc, d, p):
    # out[b, :] = x4flat[b] @ Wf + bf
    # lhsT = x4T K-chunk [128, B_LOC] (from x4p planes), rhs = Wf K-chunk
    # [128, LATENT]: wide-N accumulating matmuls (N=2 back-to-back
    # accumulation into the same PSUM columns loses updates on HW).
    fps = p["pool_ps"].tile([B_LOC, LATENT], F32, tag="pool")
    xt = p["fin"].tile([128, 4 * 256], F32, tag="x4T")
    for pl in range(4):
        nc.sync.dma_start(xt[:, pl * 256:(pl + 1) * 256], d["x4p"][pl])
    xtb = p["fin"].tile([128, 4 * 256], BF16, tag="x4Tb")
    nc.vector.tensor_copy(xtb[:], xt[:])
    bias_t = p["fin"].tile([B_LOC, LATENT], F32, tag="bf")
    for b in range(B_LOC):
        nc.sync.dma_start(bias_t[b: b + 1, :], d["bfv"][:].rearrange("l o -> o l"))
    xtb4 = xtb[:].rearrange("c (hb v) -> c v hb", hb=4)
    n_k = VERTS[4] * CH[4] // 128  # 512 K-chunks; 8 per Wf DMA
    FQ = 8
    for q in range(n_k // FQ):
        wt = p["wfq"].tile([128, FQ * LATENT], BF16, tag="wfq")
        nc.sync.dma_start(
            wt[:].rearrange("p (f l) -> p f l", f=FQ),
            d["Wfb"][q * FQ * 128:(q + 1) * FQ * 128, :]
            .rearrange("(f p) l -> p f l", p=128))
        for f in range(FQ):
            kc = q * FQ + f
            v, h = divmod(kc, 2)
            nc.tensor.matmul(
                fps[:, :],
                lhsT=xtb4[:, v, 2 * h: 2 * h + 2],
                rhs=wt[:, f * LATENT:(f + 1) * LATENT],
                start=(kc == 0), stop=(kc == n_k - 1))
    osb = p["fin"].tile([B_LOC, LATENT], F32, tag="osb")
    nc.vector.tensor_add(osb[:], fps[:], bias_t[:])
    nc.sync.dma_start(d["out"][:], osb[:])


def kernel(**inputs) -> np.ndarray:
    levels = _host_prep(inputs)
    nc = _build_bass(levels)
    in_maps = [_build_in_map(levels, inputs, c) for c in range(N_CORES)]
    res = run_bass_kernel_spmd(nc, in_maps, core_ids=list(range(N_CORES)))
    return np.concatenate([res.results[c]["out"] for c in range(N_CORES)],
                          axis=0).astype(np.float32)


if __name__ == "__main__":
    sys.path.insert(0, "/root/problem")
    import reference
    inp = {k: np.asarray(v) for k, v in reference.setup_inputs().items()}
    got = kernel(**inp)
    exp = np.asarray(reference.reference(**inp))
    print("rel err:", np.abs(got - exp).max() / np.abs(exp).max())


# revision 24
# speedup vs baseline: 1.1865x; 1.1865x over previous
"""GNN spiral-conv encoder on 8 TRN2 NeuronCores (Bass/Tile).

Sharding: data-parallel over batch (2 of 16 samples per core); all index
structures replicated. Final Linear uses bf16 Wf replicated per core.

Per level: per-slot f32 x tables in HBM; dma_gather pulls, per used
destination vertex and spiral slot, the source row into [128 dest, elem]
slabs; PE transpose-matmuls assemble the conv rhs in PSUM; matmuls vs
host-built block-diagonal weight chunks accumulate; bias+ELU
(= Relu(z+b) + exp(min(z+b,0)) - 1) splits across DVE and the otherwise
idle ACT engine. h rows are transposed back (PE) into f32 HBM row tables.
(A transpose=True dma_gather path that would skip the PE assembly is
gated off: it crashes this HW/ucode build; see the new_conv flag.)

Level 0's h table packs two h rows per 512B table row (chunk-half pairing:
dest j and j+64 of each 128-chunk share a row) so pool-0 indices fit int16
in one pass; the pool then runs two masked matmuls per slab (half A/B
banded value matrices).

Pools: entries sorted by dest; window gathers are issued one ahead
(software pipeline, bufs=3) instead of on demand; PE matmul vs banded
value matrix accumulates x_{i+1}^T in PSUM; levels 1-3 keep the legacy
f32 tables and (for 2/3) the legacy assembly conv. Level-3 pool stores
the x4^T planes directly in the final matmul's layout (no transpose).
"""
import sys

sys.path.insert(0, "/opt/trn_rl_repo")

import numpy as np

import concourse.bass as bass
import concourse.tile as tile
from concourse import bacc, mybir
from concourse.bass_utils import run_bass_kernel_spmd
from concourse.library_config import mlp as _mlp_lib
from concourse.masks import make_identity

F32 = mybir.dt.float32
BF16 = mybir.dt.bfloat16
I16 = mybir.dt.int16
AF = mybir.ActivationFunctionType

VERTS = [65536, 16384, 4096, 1024, 256]
SEQ = 9
CH = [3, 32, 64, 128, 256]
LATENT = 256
B = 16
N_CORES = 8
B_LOC = B // N_CORES

SGC = [2048, 2048, 1024, 512]   # spiral gather idxs per instruction
PGC = [1024, 1024, 1024, 256]    # pool gather idxs per instruction
RWIN = 64                        # pool value-matrix r-window
RBLOCK = 512                     # pool PSUM r-block
HALF = 32768


def _wrap_idx16(idx, chunk):
    """[128, n/16] int16 dma_gather layout: within each `chunk` window,
    index i -> partition i%16, col i//16; replicated to all 8 groups."""
    idx = np.asarray(idx, dtype=np.int64)
    n = idx.shape[0]
    assert n % chunk == 0 and idx.max() < HALF and idx.min() >= 0
    nin = n // chunk
    w = idx.reshape(nin, chunk // 16, 16).astype(np.int16)
    blocks = [w[j].T for j in range(nin)]
    one = np.concatenate(blocks, axis=1)  # [16, n/16]
    return np.tile(one, (8, 1))           # [128, n/16]


def _pad_to(a, n, fill=0):
    out = np.full((n,) + a.shape[1:], fill, dtype=a.dtype)
    out[: a.shape[0]] = a
    return out


def _h0_row_half(hrow):
    """Level-0 paired h-table position for h row index (1-based; 0 = zero
    row). Dest pos p = hrow-1; chunk base cb = (p//128)*128; j = p%128;
    row = cb//2 + (j%64) + 1, half = j//64."""
    if hrow == 0:
        return 0, 0
    p = hrow - 1
    cb, j = divmod(p, 128)
    return cb * 64 + (j % 64) + 1, j // 64


def _build_level_host(i, idx, col, row, val, W, b):
    N_in, N_out = VERTS[i], VERTS[i + 1]
    C_in, C_out = CH[i], CH[i + 1]
    L = dict(N_in=N_in, N_out=N_out, C_in=C_in, C_out=C_out)
    L["xe"] = max(64, 2 * C_in)      # legacy x-table row width (f32)
    L["he"] = max(64, 2 * C_out)     # h-table row width (f32)
    M = 2 * C_out
    L["M"] = M
    L["n_mh"] = -(-M // 128)
    L["new_conv"] = False  # transpose-gather conv pending HW validation

    # used conv outputs (h rows) = unique pool source columns
    used = np.unique(col)
    U = used.shape[0]
    Upad = -(-U // SGC[i]) * SGC[i]
    L.update(used=used, U=U, Upad=Upad)

    # spiral gather index lists per slot s
    src = idx[used, :]  # [U, SEQ]
    gcols = []
    ref_lists = []
    for s in range(SEQ):
        ss = src[:, s]
        if i == 0:
            ref = np.unique(ss)
            assert ref.shape[0] < HALF, ref.shape
            loc = np.searchsorted(ref, ss) + 1
            ref_lists.append(ref)
        else:
            loc = ss + 1
        gcols.append(_wrap_idx16(_pad_to(loc.astype(np.int64), Upad), SGC[i]))
    L["gidx"] = np.concatenate(gcols, axis=1)  # [128, SEQ*Upad/16]
    L["ref_lists"] = ref_lists

    if L["new_conv"]:
        # per-slot block-diagonal weight chunks [Ks, M] (slot 0 gets a
        # bias row fed by the ones-component of its x table)
        Wbs = []
        for s in range(SEQ):
            Ks = 2 * C_in + (1 if s == 0 else 0)
            Wc = np.zeros((Ks, M), dtype=np.float32)
            for bb in range(B_LOC):
                Wc[bb * C_in:(bb + 1) * C_in,
                   bb * C_out:(bb + 1) * C_out] = W[s * C_in:(s + 1) * C_in, :]
                if s == 0:
                    Wc[2 * C_in, bb * C_out:(bb + 1) * C_out] = b
            Wbs.append(Wc)
        L["Wbs"] = Wbs
    else:
        # legacy conv K-chunks; level 0 instead packs all 9 slots into
        # one gapless 54-row chunk (strided DVE pack-copy + one PE
        # transpose per 128-dest sub-block)
        if i == 0:
            kw = 2 * C_in
            Wc = np.zeros((SEQ * kw, M), dtype=np.float32)
            for s in range(SEQ):
                for bb in range(B_LOC):
                    Wc[s * kw + bb * C_in: s * kw + (bb + 1) * C_in,
                       bb * C_out:(bb + 1) * C_out] = \
                        W[s * C_in:(s + 1) * C_in, :]
            L.update(kchunks=None, Wcs=[Wc])
        else:
            if 2 * C_in <= 128:
                kwidth = 2 * C_in
                raw = [(s, None) for s in range(SEQ)]
            else:
                kwidth = C_in
                raw = [(s, bb) for s in range(SEQ) for bb in range(B_LOC)]
            if kwidth <= 32:
                offs = [0, 32, 64]
            elif kwidth <= 64:
                offs = [0, 64]
            else:
                offs = [0]
            cs = len(offs)
            kchunks = []
            for j in range(0, len(raw), cs):
                kchunks.append([(s, bsel, kwidth, offs[t])
                                for t, (s, bsel) in enumerate(raw[j: j + cs])])
            Wcs = []
            for ck in kchunks:
                K = max(poff + kwidth for (_, _, _, poff) in ck)
                Wc = np.zeros((K, M), dtype=np.float32)
                for (s, bsel, kw, poff) in ck:
                    if bsel is None:
                        for bb in range(B_LOC):
                            Wc[poff + bb * C_in: poff + (bb + 1) * C_in,
                               bb * C_out:(bb + 1) * C_out] = \
                                W[s * C_in:(s + 1) * C_in, :]
                    else:
                        Wc[poff: poff + C_in,
                           bsel * C_out:(bsel + 1) * C_out] = \
                            W[s * C_in:(s + 1) * C_in, :]
                Wcs.append(Wc)
            L.update(kchunks=kchunks, Wcs=Wcs)
        bias_flat = np.repeat(b[None, :], B_LOC, axis=0).reshape(-1)  # [M]
        L["bias"] = _pad_to(bias_flat.astype(np.float32), L["n_mh"] * 128) \
            .reshape(L["n_mh"], 128).T.copy()  # [128, n_mh]

    # pool: sort entries by dest row, pack into slabs
    colpos = np.searchsorted(used, col)  # h row index - 1
    order = np.argsort(row, kind="stable")
    er, ec, ev = row[order], colpos[order] + 1, val[order]
    slabs, cur = [], []
    for k in range(er.shape[0]):
        r = int(er[k])
        if cur and (len(cur) >= 128 or r - cur[0][2] >= RWIN
                    or (r // RBLOCK) != (cur[0][2] // RBLOCK)):
            slabs.append(cur)
            cur = []
        cur.append((int(ec[k]), float(ev[k]), r))
    if cur:
        slabs.append(cur)
    nslab = len(slabs)
    paired = i == 0
    n_half = 2 if paired else 1
    pool_idx = np.zeros((nslab * 128,), dtype=np.int64)
    S2 = np.zeros((128, nslab * n_half * RWIN), dtype=np.float32)
    slab_meta = []
    for si, sl in enumerate(slabs):
        r0 = sl[0][2]
        g = r0 // RBLOCK
        w_off = r0 - g * RBLOCK
        if w_off + RWIN > RBLOCK:
            w_off = RBLOCK - RWIN
        slab_meta.append((g, w_off))
        for j, (hrow, v, r) in enumerate(sl):
            wpos = r - g * RBLOCK - w_off
            if paired:
                trow, half = _h0_row_half(hrow)
                pool_idx[si * 128 + j] = trow
                S2[j, (2 * si + half) * RWIN + wpos] = v
            else:
                pool_idx[si * 128 + j] = hrow
                S2[j, si * RWIN + wpos] = v
    rblocks = []
    cur_g, start = None, 0
    for si, (g, _) in enumerate(slab_meta):
        if g != cur_g:
            if cur_g is not None:
                rblocks.append((cur_g, start, si))
            cur_g, start = g, si
    if cur_g is not None:
        rblocks.append((cur_g, start, nslab))
    covered = {g for (g, _, _) in rblocks}
    for g in range(-(-N_out // RBLOCK)):
        if g not in covered:
            rblocks.append((g, 0, 0))
    rblocks.sort()
    npad = -(-nslab * 128 // PGC[i]) * PGC[i]
    L["pidx"] = _wrap_idx16(_pad_to(pool_idx, npad), PGC[i])
    L.update(nslab=nslab, paired=paired, S2=S2, slab_meta=slab_meta,
             rblocks=rblocks, npad=npad)
    return L


def _host_prep(inputs):
    g = lambda k: np.asarray(inputs[k])
    return [
        _build_level_host(i, g(f"idx{i}").astype(np.int64),
                          g(f"col{i}").astype(np.int64),
                          g(f"row{i}").astype(np.int64),
                          g(f"val{i}").astype(np.float32),
                          g(f"W{i}").astype(np.float32),
                          g(f"b{i}").astype(np.float32))
        for i in range(4)
    ]


def _build_in_map(levels, inputs, core):
    import ml_dtypes
    x = np.asarray(inputs["x"], dtype=np.float32)
    xs = x[core * B_LOC:(core + 1) * B_LOC]  # [2, N0, 3]
    m = {}
    L0 = levels[0]
    for s in range(SEQ):
        ref = L0["ref_lists"][s]
        if L0["new_conv"]:
            t = np.zeros((HALF, 128), dtype=ml_dtypes.bfloat16)
            for bb in range(B_LOC):
                t[1:1 + ref.shape[0], bb * CH[0]:(bb + 1) * CH[0]] = xs[bb][ref, :]
            if s == 0:
                t[1:1 + ref.shape[0], 2 * CH[0]] = 1.0  # bias ones-component
        else:
            t = np.zeros((HALF, 64), dtype=np.float32)
            for bb in range(B_LOC):
                t[1:1 + ref.shape[0], bb * CH[0]:(bb + 1) * CH[0]] = xs[bb][ref, :]
        m[f"x0t{s}"] = t
    for i, L in enumerate(levels):
        m[f"gidx{i}"] = L["gidx"]
        m[f"pidx{i}"] = L["pidx"]
        m[f"S2_{i}"] = L["S2"]
        if L["new_conv"]:
            for s, Wc in enumerate(L["Wbs"]):
                m[f"Wb{i}_{s}"] = Wc
        else:
            for k, Wc in enumerate(L["Wcs"]):
                m[f"W{i}_{k}"] = Wc
            m[f"bias{i}"] = L["bias"]
    m["Wfb"] = np.asarray(inputs["Wf"], dtype=np.float32).astype(ml_dtypes.bfloat16)
    m["bfv"] = np.asarray(inputs["bf"], dtype=np.float32)[:, None]
    return m


def _build_bass(levels):
    nc = bacc.Bacc("TRN2", target_bir_lowering=False, debug=False,
                   num_devices=N_CORES, num_swdge_queues=4)
    d = {}
    x0w, x0dt = ((128, BF16) if levels[0]["new_conv"] else (64, F32))
    for s in range(SEQ):
        d[f"x0t{s}"] = nc.dram_tensor(f"x0t{s}", [HALF, x0w], x0dt,
                                      kind="ExternalInput")
    for i, L in enumerate(levels):
        d[f"gidx{i}"] = nc.dram_tensor(
            f"gidx{i}", [128, SEQ * L["Upad"] // 16], I16, kind="ExternalInput")
        d[f"pidx{i}"] = nc.dram_tensor(
            f"pidx{i}", [128, L["npad"] // 16], I16, kind="ExternalInput")
        n_half = 2 if L["paired"] else 1
        d[f"S2_{i}"] = nc.dram_tensor(
            f"S2_{i}", [128, L["nslab"] * n_half * RWIN], F32,
            kind="ExternalInput")
        if L["new_conv"]:
            for s, Wc in enumerate(L["Wbs"]):
                d[f"Wb{i}_{s}"] = nc.dram_tensor(
                    f"Wb{i}_{s}", list(Wc.shape), F32, kind="ExternalInput")
        else:
            for k, Wc in enumerate(L["Wcs"]):
                d[f"W{i}_{k}"] = nc.dram_tensor(f"W{i}_{k}", list(Wc.shape),
                                                F32, kind="ExternalInput")
            d[f"bias{i}"] = nc.dram_tensor(f"bias{i}", [128, L["n_mh"]], F32,
                                           kind="ExternalInput")
        if i == 1:
            if L["new_conv"]:
                # bf16 x table, ones-component at col 2*C_in for the bias row
                d["xt1"] = nc.dram_tensor("xt1", [L["N_in"] + 1, 128], BF16,
                                          kind="Internal")
            else:
                d["xt1"] = nc.dram_tensor("xt1", [L["N_in"] + 1, L["xe"]],
                                          F32, kind="Internal")
        elif i > 1:
            d[f"xt{i}"] = nc.dram_tensor(f"xt{i}", [L["N_in"] + 1, L["xe"]],
                                         F32, kind="Internal")
        if i == 0:
            d["hta0"] = nc.dram_tensor("hta0", [L["Upad"] // 2 + 1, 128], F32,
                                       kind="Internal")
        else:
            d[f"hta{i}"] = nc.dram_tensor(f"hta{i}", [L["Upad"] + 1, L["he"]],
                                          F32, kind="Internal")
    d["x4p"] = nc.dram_tensor("x4p", [4, 128, 256], F32, kind="Internal")
    d["Wfb"] = nc.dram_tensor("Wfb", [VERTS[4] * CH[4], LATENT], BF16,
                              kind="ExternalInput")
    d["bfv"] = nc.dram_tensor("bfv", [LATENT, 1], F32, kind="ExternalInput")
    d["out"] = nc.dram_tensor("out", [B_LOC, LATENT], F32, kind="ExternalOutput")

    with tile.TileContext(nc) as tc:
        nc.gpsimd.load_library(_mlp_lib)
        _emit(nc, tc, d, levels)
    nc.compile()
    return nc


def _emit(nc, tc, d, levels):
    from contextlib import ExitStack
    import itertools
    with ExitStack() as ctx:
        p = {}
        # SWDGE sem lanes are assigned round-robin over Pool-engine DMA
        # instructions in emission order (8 lanes); queue_num must follow
        # emission order so each lane stays locked to one queue.
        p["qctr"] = itertools.count()
        p["ident"] = ctx.enter_context(tc.tile_pool(name="ident", bufs=1))
        p["w"] = ctx.enter_context(tc.tile_pool(name="wp", bufs=1))
        p["idx"] = ctx.enter_context(tc.tile_pool(name="idxp", bufs=3))
        p["g"] = ctx.enter_context(tc.tile_pool(name="gp", bufs=2))
        p["rhs"] = ctx.enter_context(tc.tile_pool(name="rhsp", bufs=3))
        p["h"] = ctx.enter_context(tc.tile_pool(name="hp", bufs=3))
        p["tmp"] = ctx.enter_context(tc.tile_pool(name="tmpp", bufs=2))
        p["nat"] = ctx.enter_context(tc.tile_pool(name="natp", bufs=2))
        p["s2"] = ctx.enter_context(tc.tile_pool(name="s2p", bufs=2))
        p["ph"] = ctx.enter_context(tc.tile_pool(name="php", bufs=3))
        p["fin"] = ctx.enter_context(tc.tile_pool(name="finp", bufs=1))
        p["wfq"] = ctx.enter_context(tc.tile_pool(name="wfqp", bufs=3))
        p["asm_ps"] = ctx.enter_context(
            tc.tile_pool(name="asmps", bufs=1, space="PSUM"))
        p["conv_ps"] = ctx.enter_context(
            tc.tile_pool(name="convps", bufs=1, space="PSUM"))
        p["nat_ps"] = ctx.enter_context(
            tc.tile_pool(name="natps", bufs=2, space="PSUM"))
        p["pool_ps"] = ctx.enter_context(
            tc.tile_pool(name="poolps", bufs=1, space="PSUM"))

        ident = p["ident"].tile([128, 128], F32)
        make_identity(nc, ident[:])
        zrow = p["ident"].tile([1, 512], F32)
        nc.vector.memset(zrow[:], 0.0)
        zrowb = p["ident"].tile([1, 128], BF16)
        nc.vector.memset(zrowb[:], 0.0)
        if levels[1]["new_conv"]:
            nc.sync.dma_start(d["xt1"][0:1, :], zrowb[:1, :])
        else:
            nc.sync.dma_start(d["xt1"][0:1, :], zrow[:1, :levels[1]["xe"]])
        for i in range(2, 4):
            nc.sync.dma_start(d[f"xt{i}"][0:1, :], zrow[:1, :levels[i]["xe"]])
        nc.sync.dma_start(d["hta0"][0:1, :], zrow[:1, :128])
        for i in range(1, 4):
            nc.sync.dma_start(d[f"hta{i}"][0:1, :], zrow[:1, :levels[i]["he"]])

        for i, L in enumerate(levels):
            if L["new_conv"]:
                _emit_conv_new(nc, d, i, L, ident, p)
            else:
                _emit_conv_legacy(nc, d, i, L, ident, p)
            _emit_pool(nc, d, levels, i, L, ident, p)
        _emit_final(nc, d, p)


def _emit_conv_new(nc, d, i, L, ident, p):
    """Levels 0/1: transpose-gather bf16 conv (no PE assembly)."""
    C_in, M = L["C_in"], L["M"]
    Upad, sgc = L["Upad"], SGC[i]
    n_gi = Upad // sgc

    Wb = []
    for s in range(SEQ):
        Ks = L["Wbs"][s].shape[0]
        wt32 = p["w"].tile([Ks, M], F32, tag=f"Wb32_{i}_{s}")
        nc.sync.dma_start(wt32[:], d[f"Wb{i}_{s}"][:])
        wtb = p["w"].tile([Ks, M], BF16, tag=f"Wb_{i}_{s}")
        nc.vector.tensor_copy(wtb[:], wt32[:])
        Wb.append(wtb)

    for gi in range(n_gi):
        gts = []
        for s in range(SEQ):
            it = p["idx"].tile([128, sgc // 16], I16, tag="gidx")
            nc.sync.dma_start(
                it[:], d[f"gidx{i}"][:, (s * Upad + gi * sgc) // 16:
                                     (s * Upad + (gi + 1) * sgc) // 16])
            gt = p["g"].tile([128, sgc], BF16, tag=f"g{s}")
            src_ap = d[f"x0t{s}"][:] if i == 0 else d["xt1"][:]
            nc.gpsimd.dma_gather(
                gt[:].rearrange("p (o n) -> p o n", o=1),
                src_ap, it[:], sgc, sgc, 128,
                transpose=True, queue_num=next(p["qctr"]) % 4)
            gts.append(gt)
        for blk in range(sgc // 512):
            if blk % 2 == 0:
                cps = p["conv_ps"].tile([128, 512], F32, tag="conv")
            else:
                cps = p["asm_ps"].tile([128, 512], F32, tag="asm")
            for s in range(SEQ):
                Ks = L["Wbs"][s].shape[0]
                nc.tensor.matmul(
                    cps[:M, :],
                    lhsT=Wb[s][:Ks, :M],
                    rhs=gts[s][:Ks, blk * 512:(blk + 1) * 512],
                    start=(s == 0), stop=(s == SEQ - 1))
            # ELU: h = Relu(z) + exp(min(z, 0)) - 1
            tneg = p["tmp"].tile([128, 512], F32, tag="tneg")
            nc.vector.tensor_scalar_min(tneg[:M, :], cps[:M, :], 0.0)
            nc.scalar.activation(tneg[:M, :], tneg[:M, :], AF.Exp)
            hT = p["h"].tile([128, 512], F32, tag="hT")
            nc.scalar.activation(hT[:M, :], cps[:M, :], AF.Relu)
            nc.vector.tensor_add(hT[:M, :], hT[:M, :], tneg[:M, :])
            nc.vector.tensor_scalar_add(hT[:M, :], hT[:M, :], -1.0)
            # transpose 128-dest chunks and store h rows
            for cc in range(4):
                nps = p["nat_ps"].tile([128, 128], F32, tag="tp")
                nc.tensor.matmul(nps[:, :M],
                                 lhsT=hT[:M, cc * 128:(cc + 1) * 128],
                                 rhs=ident[:M, :M], is_transpose=True,
                                 start=True, stop=True)
                nsb = p["nat"].tile([128, 128], F32, tag="hnatsb")
                nc.vector.tensor_copy(nsb[:, :M], nps[:, :M])
                pos0 = gi * sgc + blk * 512 + cc * 128
                if i == 0:
                    t0 = pos0 // 2 + 1
                    nc.sync.dma_start(d["hta0"][t0: t0 + 64, 0:64],
                                      nsb[0:64, :64])
                    nc.sync.dma_start(d["hta0"][t0: t0 + 64, 64:128],
                                      nsb[64:128, :64])
                else:
                    nc.sync.dma_start(
                        d[f"hta{i}"][pos0 + 1: pos0 + 129, :M], nsb[:, :M])


def _emit_conv_legacy(nc, d, i, L, ident, p):
    C_in, C_out, M, n_mh = L["C_in"], L["C_out"], L["M"], L["n_mh"]
    xe = L["xe"]
    Upad = L["Upad"]
    sgc = SGC[i]
    kchunks = L["kchunks"]

    Wts = []
    for k, Wc in enumerate(L["Wcs"]):
        wt = p["w"].tile([Wc.shape[0], Wc.shape[1]], F32, tag=f"W{i}_{k}")
        nc.sync.dma_start(wt[:], d[f"W{i}_{k}"][:])
        Wts.append(wt)
    bias_t = p["w"].tile([128, n_mh], F32, tag=f"bias{i}")
    nc.sync.dma_start(bias_t[:], d[f"bias{i}"][:])

    n_gi = Upad // sgc
    nslab_g = sgc // 128
    packed = i == 0  # gapless 54-row chunk via DVE pack + one transpose
    sw = nslab_g * xe  # per-slot region width (1024 at every level)

    for gi in range(n_gi):
        gbig = p["g"].tile([128, SEQ * sw], F32, tag="gbig")
        gtiles = [gbig[:, s * sw:(s + 1) * sw] for s in range(SEQ)]
        for s in range(SEQ):
            it = p["idx"].tile([128, sgc // 16], I16, tag="gidx")
            nc.sync.dma_start(
                it[:], d[f"gidx{i}"][:, (s * Upad + gi * sgc) // 16:
                                     (s * Upad + (gi + 1) * sgc) // 16])
            src_ap = d[f"x0t{s}"][:] if i == 0 else d[f"xt{i}"][:]
            nc.gpsimd.dma_gather(
                gtiles[s].rearrange("p (n e) -> p n e", e=xe),
                src_ap, it[:], sgc, sgc, xe,
                single_packet=False, queue_num=next(p["qctr"]) % 4)
        if packed:
            kw2 = 2 * C_in           # 6
            Kp = SEQ * kw2           # 54
            g4 = gbig[:].rearrange("p (s w e) -> p s w e",
                                   s=SEQ, w=nslab_g)
            for blk in range(nslab_g // 4):
                cps = p["conv_ps"].tile([128, 512 * n_mh], F32, tag="conv")
                aps = p["asm_ps"].tile([128, 512], F32, tag="asm")
                for sub in range(4):
                    slab_i = blk * 4 + sub
                    pk = p["rhs"].tile([128, 64], F32, tag="pk")
                    nc.vector.tensor_copy(
                        pk[:, :Kp].rearrange("p (s o e) -> p s o e",
                                             s=SEQ, o=1),
                        g4[:, :, slab_i: slab_i + 1, 0: kw2])
                    nc.tensor.matmul(
                        aps[0: Kp, sub * 128:(sub + 1) * 128],
                        lhsT=pk[:, :Kp], rhs=ident[:],
                        is_transpose=True, start=True, stop=True)
                rb = p["rhs"].tile([128, 512], F32, tag="rhs")
                nc.vector.tensor_copy(rb[:Kp, :], aps[:Kp, :])
                nc.tensor.matmul(
                    cps[:M, :512], lhsT=Wts[0][:, :M],
                    rhs=rb[:Kp, :], start=True, stop=True)
                _emit_elu_store(nc, d, i, L, ident, p, bias_t, cps, gi, blk)
            continue
        for blk in range(nslab_g // 4):
            cps = p["conv_ps"].tile([128, 512 * n_mh], F32, tag="conv")
            for kci, ck in enumerate(kchunks):
                K = max(poff + kw for (_, _, kw, poff) in ck)
                aps = p["asm_ps"].tile([128, 512], F32, tag="asm")
                for sub in range(4):
                    slab_i = blk * 4 + sub
                    for (s, bsel, kw, poff) in ck:
                        c0 = slab_i * xe + (0 if bsel is None
                                            else bsel * C_in)
                        in_ap = gtiles[s][:, c0: c0 + kw]
                        nc.tensor.matmul(
                            aps[poff: poff + kw,
                                sub * 128:(sub + 1) * 128],
                            lhsT=in_ap, rhs=ident[:],
                            start=True, stop=True)
                rb = p["rhs"].tile([128, 512], F32, tag="rhs")
                nc.vector.tensor_copy(rb[:K, :], aps[:K, :])
                for mh in range(n_mh):
                    mw = min(128, M - mh * 128)
                    nc.tensor.matmul(
                        cps[:mw, mh * 512:(mh + 1) * 512],
                        lhsT=Wts[kci][:, mh * 128: mh * 128 + mw],
                        rhs=rb[:K, :], start=(kci == 0),
                        stop=(kci == len(kchunks) - 1))
            _emit_elu_store(nc, d, i, L, ident, p, bias_t, cps, gi, blk)


def _emit_elu_store(nc, d, i, L, ident, p, bias_t, cps, gi, blk):
    M, n_mh = L["M"], L["n_mh"]
    sgc = SGC[i]
    for mh in range(n_mh):
        mw = min(128, M - mh * 128)
        # bias + ELU: h = Relu(z+b) + exp(min(z+b, 0)) - 1
        hT = p["h"].tile([128, 512], F32, tag="hT")
        tneg = p["tmp"].tile([128, 512], F32, tag="tneg")
        bsl = bias_t[:mw, mh: mh + 1]
        csl = cps[:mw, mh * 512:(mh + 1) * 512]
        nc.vector.tensor_scalar(tneg[:mw, :], csl, bsl, 0.0,
                                mybir.AluOpType.add,
                                mybir.AluOpType.min)
        nc.scalar.activation(tneg[:mw, :], tneg[:mw, :], AF.Exp)
        nc.scalar.activation(hT[:mw, :], csl, AF.Relu, bias=bsl)
        nc.vector.tensor_add(hT[:mw, :], hT[:mw, :], tneg[:mw, :])
        nc.vector.tensor_scalar_add(hT[:mw, :], hT[:mw, :], -1.0)
        # naturalize 128-dest chunks and store h rows
        for cc in range(4):
            nps = p["nat_ps"].tile([128, 128], F32, tag="tp")
            nc.tensor.matmul(nps[:, :mw],
                             lhsT=hT[:mw, cc * 128:(cc + 1) * 128],
                             rhs=ident[:mw, :mw], is_transpose=True,
                             start=True, stop=True)
            nsb = p["nat"].tile([128, 128], F32, tag="hnatsb")
            nc.vector.tensor_copy(nsb[:, :mw], nps[:, :mw])
            pos0 = gi * sgc + blk * 512 + cc * 128
            if i == 0:
                t0 = pos0 // 2 + 1
                nc.sync.dma_start(d["hta0"][t0: t0 + 64, 0:64],
                                  nsb[0:64, :64])
                nc.sync.dma_start(d["hta0"][t0: t0 + 64, 64:128],
                                  nsb[64:128, :64])
            else:
                nc.sync.dma_start(
                    d[f"hta{i}"][pos0 + 1: pos0 + 129,
                                 mh * 128: mh * 128 + mw],
                    nsb[:, :mw])


def _emit_pool(nc, d, levels, i, L, ident, p):
    nslab, npad = L["nslab"], L["npad"]
    paired = L["paired"]
    n_half = 2 if paired else 1
    pgc = PGC[i]
    helem = 128 if paired else L["he"]
    M_next = 512 if i == 3 else (2 * levels[i + 1]["C_in"])
    n_mh_next = -(-M_next // 128)
    win_slabs = pgc // 128
    n_win = npad // pgc
    tiles = {}

    def issue(wi):
        it = p["idx"].tile([128, pgc // 16], I16, tag="pidx")
        nc.sync.dma_start(
            it[:], d[f"pidx{i}"][:, wi * pgc // 16: (wi + 1) * pgc // 16])
        gt = p["ph"].tile([128, win_slabs * helem], F32, tag="ph")
        nc.gpsimd.dma_gather(
            gt[:].rearrange("p (n e) -> p n e", e=helem),
            d[f"hta{i}"][:], it[:], pgc, pgc, helem,
            single_packet=False, queue_num=next(p["qctr"]) % 4)
        tiles[wi] = gt

    for wi in range(min(2, n_win)):
        issue(wi)

    xt1_bf = i == 0 and levels[1]["new_conv"]
    if xt1_bf:
        xsb_a = p["nat"].tile([128, 128], BF16, tag="xsb0")
        xsb_b = p["nat"].tile([128, 128], BF16, tag="xsb0")
        xsb = [xsb_a, xsb_b]
        for t in xsb:
            nc.vector.memset(t[:], 0.0)
            nc.vector.memset(t[:, 64:65], 1.0)

    for (g, s0, s1) in L["rblocks"]:
        n_rc = min(RBLOCK, L["N_out"] - g * RBLOCK)
        s2t = None
        if s1 > s0:
            s2t = p["s2"].tile([128, (s1 - s0) * n_half * RWIN], F32, tag="s2")
            nc.sync.dma_start(
                s2t[:], d[f"S2_{i}"][:, s0 * n_half * RWIN:
                                     s1 * n_half * RWIN])
        for mh in range(n_mh_next):
            mw = min(128, M_next - mh * 128)
            pps = p["pool_ps"].tile([128, RBLOCK], F32, tag="pool")
            nc.vector.memset(pps[:mw, :], 0.0)
            for si in range(s0, s1):
                wi, sub = divmod(si, win_slabs)
                if wi + 1 < n_win and wi + 1 not in tiles:
                    issue(wi + 1)
                (_, w_off) = L["slab_meta"][si]
                gt = tiles[wi]
                for hf in range(n_half):
                    nc.tensor.matmul(
                        pps[:mw, w_off: w_off + RWIN],
                        lhsT=gt[:, sub * helem + hf * 64 + mh * 128:
                                sub * helem + hf * 64 + mh * 128 + mw]
                        if paired else
                        gt[:, sub * helem + mh * 128:
                           sub * helem + mh * 128 + mw],
                        rhs=s2t[:, ((si - s0) * n_half + hf) * RWIN:
                                ((si - s0) * n_half + hf + 1) * RWIN],
                        start=False,
                        stop=(si == s1 - 1 and hf == n_half - 1),
                        skip_group_check=True)
            xTs = p["nat"].tile([128, RBLOCK], F32, tag="xT")
            nc.vector.tensor_copy(xTs[:mw, :], pps[:mw, :])
            if i == 3:
                # store x4^T plane directly in final-matmul layout
                # plane index p = 2h + b for mh = 2b + h
                b_, h_ = divmod(mh, 2)
                nc.sync.dma_start(d["x4p"][2 * h_ + b_],
                                  xTs[:mw, :256])
                continue
            for cc in range(-(-n_rc // 128)):
                ncc = min(128, n_rc - cc * 128)
                nps = p["nat_ps"].tile([128, 128], F32, tag="tp")
                nc.tensor.matmul(nps[:ncc, :mw],
                                 lhsT=xTs[:mw, cc * 128: cc * 128 + ncc],
                                 rhs=ident[:mw, :mw], is_transpose=True,
                                 start=True, stop=True)
                row0 = g * RBLOCK + cc * 128 + 1
                if xt1_bf:
                    nsb = xsb[cc % 2]
                    nc.vector.tensor_copy(nsb[:ncc, :64], nps[:ncc, :64])
                    nc.sync.dma_start(
                        d["xt1"][row0: row0 + ncc, :], nsb[:ncc, :])
                else:
                    nsb = p["nat"].tile([128, 128], F32, tag="xnatsb")
                    nc.vector.tensor_copy(nsb[:ncc, :mw], nps[:ncc, :mw])
                    nc.sync.dma_start(
                        d[f"xt{i + 1}"][row0: row0 + ncc,
                                        mh * 128: mh * 128 + mw],
                        nsb[:ncc, :mw])


def _emit_final(nc, d, p):
    # out[b, :] = x4flat[b] @ Wf + bf
    # lhsT = x4T K-chunk [128, B_LOC] (from x4p planes), rhs = Wf K-chunk
    # [128, LATENT]: wide-N accumulating matmuls (N=2 back-to-back
    # accumulation into the same PSUM columns loses updates on HW).
    fps = p["pool_ps"].tile([B_LOC, LATENT], F32, tag="pool")
    xt = p["fin"].tile([128, 4 * 256], F32, tag="x4T")
    for pl in range(4):
        nc.sync.dma_start(xt[:, pl * 256:(pl + 1) * 256], d["x4p"][pl])
    xtb = p["fin"].tile([128, 4 * 256], BF16, tag="x4Tb")
    nc.vector.tensor_copy(xtb[:], xt[:])
    bias_t = p["fin"].tile([B_LOC, LATENT], F32, tag="bf")
    for b in range(B_LOC):
        nc.sync.dma_start(bias_t[b: b + 1, :], d["bfv"][:].rearrange("l o -> o l"))
    xtb4 = xtb[:].rearrange("c (hb v) -> c v hb", hb=4)
    n_k = VERTS[4] * CH[4] // 128  # 512 K-chunks; 4 per Wf DMA
    for q in range(n_k // 4):
        wt = p["wfq"].tile([128, 4 * LATENT], BF16, tag="wfq")
        nc.sync.dma_start(
            wt[:].rearrange("p (f l) -> p f l", f=4),
            d["Wfb"][q * 512:(q + 1) * 512, :]
            .rearrange("(f p) l -> p f l", p=128))
        for f in range(4):
            kc = q * 4 + f
            v, h = divmod(kc, 2)
            nc.tensor.matmul(
                fps[:, :],
                lhsT=xtb4[:, v, 2 * h: 2 * h + 2],
                rhs=wt[:, f * LATENT:(f + 1) * LATENT],
                start=(kc == 0), stop=(kc == n_k - 1))
    osb = p["fin"].tile([B_LOC, LATENT], F32, tag="osb")
    nc.vector.tensor_add(osb[:], fps[:], bias_t[:])
    nc.sync.dma_start(d["out"][:], osb[:])


def kernel(**inputs) -> np.ndarray:
    levels = _host_prep(inputs)
    nc = _build_bass(levels)
    in_maps = [_build_in_map(levels, inputs, c) for c in range(N_CORES)]
    res = run_bass_kernel_spmd(nc, in_maps, core_ids=list(range(N_CORES)))
    return np.concatenate([res.results[c]["out"] for c in range(N_CORES)],
                          axis=0).astype(np.float32)


if __name__ == "__main__":
    sys.path.insert(0, "/root/problem")
    import reference
    inp = {k: np.asarray(v) for k, v in reference.setup_inputs().items()}
    got = kernel(**inp)
    exp = np.asarray(reference.reference(**inp))
    print("rel err:", np.abs(got - exp).max() / np.abs(exp).max())


# revision 26
# speedup vs baseline: 1.4080x; 1.1867x over previous
"""GNN spiral-conv encoder on 8 TRN2 NeuronCores (Bass/Tile).

Sharding: data-parallel over batch (2 of 16 samples per core); all index
structures replicated. Final Linear uses bf16 Wf replicated per core.

Per level: per-slot f32 x tables in HBM; dma_gather pulls, per used
destination vertex and spiral slot, the source row into [128 dest, elem]
slabs; PE transpose-matmuls assemble the conv rhs in PSUM; matmuls vs
host-built block-diagonal weight chunks accumulate; bias+ELU
(= Relu(z+b) + exp(min(z+b,0)) - 1) splits across DVE and the otherwise
idle ACT engine. h rows are transposed back (PE) into f32 HBM row tables.
(A transpose=True dma_gather path that would skip the PE assembly is
gated off: it crashes this HW/ucode build; see the new_conv flag.)

Level 0's h table packs two h rows per 512B table row (chunk-half pairing:
dest j and j+64 of each 128-chunk share a row) so pool-0 indices fit int16
in one pass; the pool then runs two masked matmuls per slab (half A/B
banded value matrices).

Pools: entries sorted by dest; window gathers are issued one ahead
(software pipeline, bufs=3) instead of on demand; PE matmul vs banded
value matrix accumulates x_{i+1}^T in PSUM; levels 1-3 keep the legacy
f32 tables and (for 2/3) the legacy assembly conv. Level-3 pool stores
the x4^T planes directly in the final matmul's layout (no transpose).
"""
import sys

sys.path.insert(0, "/opt/trn_rl_repo")

import numpy as np

import concourse.bass as bass
import concourse.tile as tile
from concourse import bacc, mybir
from concourse.bass_utils import run_bass_kernel_spmd
from concourse.library_config import mlp as _mlp_lib
from concourse.masks import make_identity

F32 = mybir.dt.float32
BF16 = mybir.dt.bfloat16
I16 = mybir.dt.int16
AF = mybir.ActivationFunctionType

VERTS = [65536, 16384, 4096, 1024, 256]
SEQ = 9
CH = [3, 32, 64, 128, 256]
LATENT = 256
B = 16
N_CORES = 8
B_LOC = B // N_CORES

SGC = [2048, 2048, 1024, 512]   # spiral gather idxs per instruction
PGC = [1024, 1024, 1024, 256]    # pool gather idxs per instruction
RWIN = 64                        # pool value-matrix r-window
RBLOCK = 512                     # pool PSUM r-block
HALF = 32768


def _wrap_idx16(idx, chunk):
    """[128, n/16] int16 dma_gather layout: within each `chunk` window,
    index i -> partition i%16, col i//16; replicated to all 8 groups."""
    idx = np.asarray(idx, dtype=np.int64)
    n = idx.shape[0]
    assert n % chunk == 0 and idx.max() < HALF and idx.min() >= 0
    nin = n // chunk
    w = idx.reshape(nin, chunk // 16, 16).astype(np.int16)
    blocks = [w[j].T for j in range(nin)]
    one = np.concatenate(blocks, axis=1)  # [16, n/16]
    return np.tile(one, (8, 1))           # [128, n/16]


def _pad_to(a, n, fill=0):
    out = np.full((n,) + a.shape[1:], fill, dtype=a.dtype)
    out[: a.shape[0]] = a
    return out


def _h0_row_half(hrow):
    """Level-0 paired h-table position for h row index (1-based; 0 = zero
    row). Dest pos p = hrow-1; chunk base cb = (p//128)*128; j = p%128;
    row = cb//2 + (j%64) + 1, half = j//64."""
    if hrow == 0:
        return 0, 0
    p = hrow - 1
    cb, j = divmod(p, 128)
    return cb * 64 + (j % 64) + 1, j // 64


def _build_level_host(i, idx, col, row, val, W, b):
    N_in, N_out = VERTS[i], VERTS[i + 1]
    C_in, C_out = CH[i], CH[i + 1]
    L = dict(N_in=N_in, N_out=N_out, C_in=C_in, C_out=C_out)
    L["xe"] = max(64, 2 * C_in)      # legacy x-table row width (f32)
    L["he"] = max(64, 2 * C_out)     # h-table row width (f32)
    M = 2 * C_out
    L["M"] = M
    L["n_mh"] = -(-M // 128)
    L["new_conv"] = False  # transpose-gather conv pending HW validation

    # used conv outputs (h rows) = unique pool source columns
    used = np.unique(col)
    U = used.shape[0]
    Upad = -(-U // SGC[i]) * SGC[i]
    L.update(used=used, U=U, Upad=Upad)

    # spiral gather index lists per slot s
    src = idx[used, :]  # [U, SEQ]
    gcols = []
    ref_lists = []
    for s in range(SEQ):
        ss = src[:, s]
        if i == 0:
            ref = np.unique(ss)
            assert ref.shape[0] < HALF, ref.shape
            loc = np.searchsorted(ref, ss) + 1
            ref_lists.append(ref)
        else:
            loc = ss + 1
        gcols.append(_wrap_idx16(_pad_to(loc.astype(np.int64), Upad), SGC[i]))
    L["gidx"] = np.concatenate(gcols, axis=1)  # [128, SEQ*Upad/16]
    L["ref_lists"] = ref_lists

    if L["new_conv"]:
        # per-slot block-diagonal weight chunks [Ks, M] (slot 0 gets a
        # bias row fed by the ones-component of its x table)
        Wbs = []
        for s in range(SEQ):
            Ks = 2 * C_in + (1 if s == 0 else 0)
            Wc = np.zeros((Ks, M), dtype=np.float32)
            for bb in range(B_LOC):
                Wc[bb * C_in:(bb + 1) * C_in,
                   bb * C_out:(bb + 1) * C_out] = W[s * C_in:(s + 1) * C_in, :]
                if s == 0:
                    Wc[2 * C_in, bb * C_out:(bb + 1) * C_out] = b
            Wbs.append(Wc)
        L["Wbs"] = Wbs
    else:
        # legacy conv K-chunks; level 0 instead packs all 9 slots into
        # one gapless 54-row chunk (strided DVE pack-copy + one PE
        # transpose per 128-dest sub-block)
        if i == 0:
            kw = 2 * C_in
            Wc = np.zeros((SEQ * kw, M), dtype=np.float32)
            for s in range(SEQ):
                for bb in range(B_LOC):
                    Wc[s * kw + bb * C_in: s * kw + (bb + 1) * C_in,
                       bb * C_out:(bb + 1) * C_out] = \
                        W[s * C_in:(s + 1) * C_in, :]
            L.update(kchunks=None, Wcs=[Wc])
        else:
            if 2 * C_in <= 128:
                kwidth = 2 * C_in
                raw = [(s, None) for s in range(SEQ)]
            else:
                kwidth = C_in
                raw = [(s, bb) for s in range(SEQ) for bb in range(B_LOC)]
            if kwidth <= 32:
                offs = [0, 32, 64]
            elif kwidth <= 64:
                offs = [0, 64]
            else:
                offs = [0]
            cs = len(offs)
            kchunks = []
            for j in range(0, len(raw), cs):
                kchunks.append([(s, bsel, kwidth, offs[t])
                                for t, (s, bsel) in enumerate(raw[j: j + cs])])
            Wcs = []
            for ck in kchunks:
                K = max(poff + kwidth for (_, _, _, poff) in ck)
                Wc = np.zeros((K, M), dtype=np.float32)
                for (s, bsel, kw, poff) in ck:
                    if bsel is None:
                        for bb in range(B_LOC):
                            Wc[poff + bb * C_in: poff + (bb + 1) * C_in,
                               bb * C_out:(bb + 1) * C_out] = \
                                W[s * C_in:(s + 1) * C_in, :]
                    else:
                        Wc[poff: poff + C_in,
                           bsel * C_out:(bsel + 1) * C_out] = \
                            W[s * C_in:(s + 1) * C_in, :]
                Wcs.append(Wc)
            L.update(kchunks=kchunks, Wcs=Wcs)
        bias_flat = np.repeat(b[None, :], B_LOC, axis=0).reshape(-1)  # [M]
        L["bias"] = _pad_to(bias_flat.astype(np.float32), L["n_mh"] * 128) \
            .reshape(L["n_mh"], 128).T.copy()  # [128, n_mh]

    # pool: sort entries by dest row, pack into slabs
    colpos = np.searchsorted(used, col)  # h row index - 1
    order = np.argsort(row, kind="stable")
    er, ec, ev = row[order], colpos[order] + 1, val[order]
    slabs, cur = [], []
    for k in range(er.shape[0]):
        r = int(er[k])
        if cur and (len(cur) >= 128 or r - cur[0][2] >= RWIN
                    or (r // RBLOCK) != (cur[0][2] // RBLOCK)):
            slabs.append(cur)
            cur = []
        cur.append((int(ec[k]), float(ev[k]), r))
    if cur:
        slabs.append(cur)
    nslab = len(slabs)
    paired = i == 0
    n_half = 2 if paired else 1
    pool_idx = np.zeros((nslab * 128,), dtype=np.int64)
    S2 = np.zeros((128, nslab * n_half * RWIN), dtype=np.float32)
    slab_meta = []
    for si, sl in enumerate(slabs):
        r0 = sl[0][2]
        g = r0 // RBLOCK
        w_off = r0 - g * RBLOCK
        if w_off + RWIN > RBLOCK:
            w_off = RBLOCK - RWIN
        slab_meta.append((g, w_off))
        for j, (hrow, v, r) in enumerate(sl):
            wpos = r - g * RBLOCK - w_off
            if paired:
                trow, half = _h0_row_half(hrow)
                pool_idx[si * 128 + j] = trow
                S2[j, (2 * si + half) * RWIN + wpos] = v
            else:
                pool_idx[si * 128 + j] = hrow
                S2[j, si * RWIN + wpos] = v
    rblocks = []
    cur_g, start = None, 0
    for si, (g, _) in enumerate(slab_meta):
        if g != cur_g:
            if cur_g is not None:
                rblocks.append((cur_g, start, si))
            cur_g, start = g, si
    if cur_g is not None:
        rblocks.append((cur_g, start, nslab))
    covered = {g for (g, _, _) in rblocks}
    for g in range(-(-N_out // RBLOCK)):
        if g not in covered:
            rblocks.append((g, 0, 0))
    rblocks.sort()
    npad = -(-nslab * 128 // PGC[i]) * PGC[i]
    L["pidx"] = _wrap_idx16(_pad_to(pool_idx, npad), PGC[i])
    L.update(nslab=nslab, paired=paired, S2=S2, slab_meta=slab_meta,
             rblocks=rblocks, npad=npad)
    return L


def _host_prep(inputs):
    g = lambda k: np.asarray(inputs[k])
    return [
        _build_level_host(i, g(f"idx{i}").astype(np.int64),
                          g(f"col{i}").astype(np.int64),
                          g(f"row{i}").astype(np.int64),
                          g(f"val{i}").astype(np.float32),
                          g(f"W{i}").astype(np.float32),
                          g(f"b{i}").astype(np.float32))
        for i in range(4)
    ]


def _build_in_map(levels, inputs, core):
    import ml_dtypes
    x = np.asarray(inputs["x"], dtype=np.float32)
    xs = x[core * B_LOC:(core + 1) * B_LOC]  # [2, N0, 3]
    m = {}
    L0 = levels[0]
    for s in range(SEQ):
        ref = L0["ref_lists"][s]
        if L0["new_conv"]:
            t = np.zeros((HALF, 128), dtype=ml_dtypes.bfloat16)
            for bb in range(B_LOC):
                t[1:1 + ref.shape[0], bb * CH[0]:(bb + 1) * CH[0]] = xs[bb][ref, :]
            if s == 0:
                t[1:1 + ref.shape[0], 2 * CH[0]] = 1.0  # bias ones-component
        else:
            t = np.zeros((HALF, 64), dtype=np.float32)
            for bb in range(B_LOC):
                t[1:1 + ref.shape[0], bb * CH[0]:(bb + 1) * CH[0]] = xs[bb][ref, :]
        m[f"x0t{s}"] = t
    for i, L in enumerate(levels):
        m[f"gidx{i}"] = L["gidx"]
        m[f"pidx{i}"] = L["pidx"]
        m[f"S2_{i}"] = L["S2"]
        if L["new_conv"]:
            for s, Wc in enumerate(L["Wbs"]):
                m[f"Wb{i}_{s}"] = Wc
        else:
            for k, Wc in enumerate(L["Wcs"]):
                m[f"W{i}_{k}"] = Wc
            m[f"bias{i}"] = L["bias"]
    m["Wfb"] = np.asarray(inputs["Wf"], dtype=np.float32).astype(ml_dtypes.bfloat16)
    m["bfv"] = np.asarray(inputs["bf"], dtype=np.float32)[:, None]
    return m


def _build_bass(levels):
    nc = bacc.Bacc("TRN2", target_bir_lowering=False, debug=False,
                   num_devices=N_CORES, num_swdge_queues=4)
    d = {}
    x0w, x0dt = ((128, BF16) if levels[0]["new_conv"] else (64, F32))
    for s in range(SEQ):
        d[f"x0t{s}"] = nc.dram_tensor(f"x0t{s}", [HALF, x0w], x0dt,
                                      kind="ExternalInput")
    for i, L in enumerate(levels):
        d[f"gidx{i}"] = nc.dram_tensor(
            f"gidx{i}", [128, SEQ * L["Upad"] // 16], I16, kind="ExternalInput")
        d[f"pidx{i}"] = nc.dram_tensor(
            f"pidx{i}", [128, L["npad"] // 16], I16, kind="ExternalInput")
        n_half = 2 if L["paired"] else 1
        d[f"S2_{i}"] = nc.dram_tensor(
            f"S2_{i}", [128, L["nslab"] * n_half * RWIN], F32,
            kind="ExternalInput")
        if L["new_conv"]:
            for s, Wc in enumerate(L["Wbs"]):
                d[f"Wb{i}_{s}"] = nc.dram_tensor(
                    f"Wb{i}_{s}", list(Wc.shape), F32, kind="ExternalInput")
        else:
            for k, Wc in enumerate(L["Wcs"]):
                d[f"W{i}_{k}"] = nc.dram_tensor(f"W{i}_{k}", list(Wc.shape),
                                                F32, kind="ExternalInput")
            d[f"bias{i}"] = nc.dram_tensor(f"bias{i}", [128, L["n_mh"]], F32,
                                           kind="ExternalInput")
        if i == 1:
            if L["new_conv"]:
                # bf16 x table, ones-component at col 2*C_in for the bias row
                d["xt1"] = nc.dram_tensor("xt1", [L["N_in"] + 1, 128], BF16,
                                          kind="Internal")
            else:
                d["xt1"] = nc.dram_tensor("xt1", [L["N_in"] + 1, L["xe"]],
                                          F32, kind="Internal")
        elif i > 1:
            d[f"xt{i}"] = nc.dram_tensor(f"xt{i}", [L["N_in"] + 1, L["xe"]],
                                         F32, kind="Internal")
        if i == 0:
            d["hta0"] = nc.dram_tensor("hta0", [L["Upad"] // 2 + 1, 128], F32,
                                       kind="Internal")
        else:
            d[f"hta{i}"] = nc.dram_tensor(f"hta{i}", [L["Upad"] + 1, L["he"]],
                                          F32, kind="Internal")
    d["x4p"] = nc.dram_tensor("x4p", [4, 128, 256], F32, kind="Internal")
    d["Wfb"] = nc.dram_tensor("Wfb", [VERTS[4] * CH[4], LATENT], BF16,
                              kind="ExternalInput")
    d["bfv"] = nc.dram_tensor("bfv", [LATENT, 1], F32, kind="ExternalInput")
    d["out"] = nc.dram_tensor("out", [B_LOC, LATENT], F32, kind="ExternalOutput")

    with tile.TileContext(nc) as tc:
        nc.gpsimd.load_library(_mlp_lib)
        _emit(nc, tc, d, levels)
    nc.compile()
    return nc


def _emit(nc, tc, d, levels):
    from contextlib import ExitStack
    import itertools
    with ExitStack() as ctx:
        p = {}
        # SWDGE sem lanes are assigned round-robin over Pool-engine DMA
        # instructions in emission order (8 lanes); queue_num must follow
        # emission order so each lane stays locked to one queue.
        p["qctr"] = itertools.count()
        p["ident"] = ctx.enter_context(tc.tile_pool(name="ident", bufs=1))
        p["w"] = ctx.enter_context(tc.tile_pool(name="wp", bufs=1))
        p["idx"] = ctx.enter_context(tc.tile_pool(name="idxp", bufs=9))
        p["g"] = ctx.enter_context(tc.tile_pool(name="gp", bufs=2))
        p["rhs"] = ctx.enter_context(tc.tile_pool(name="rhsp", bufs=3))
        p["h"] = ctx.enter_context(tc.tile_pool(name="hp", bufs=3))
        p["tmp"] = ctx.enter_context(tc.tile_pool(name="tmpp", bufs=2))
        p["nat"] = ctx.enter_context(tc.tile_pool(name="natp", bufs=2))
        p["s2"] = ctx.enter_context(tc.tile_pool(name="s2p", bufs=2))
        p["ph"] = ctx.enter_context(tc.tile_pool(name="php", bufs=4))
        p["fin"] = ctx.enter_context(tc.tile_pool(name="finp", bufs=1))
        p["wfq"] = ctx.enter_context(tc.tile_pool(name="wfqp", bufs=3))
        p["asm_ps"] = ctx.enter_context(
            tc.tile_pool(name="asmps", bufs=1, space="PSUM"))
        p["conv_ps"] = ctx.enter_context(
            tc.tile_pool(name="convps", bufs=1, space="PSUM"))
        p["nat_ps"] = ctx.enter_context(
            tc.tile_pool(name="natps", bufs=2, space="PSUM"))
        p["pool_ps"] = ctx.enter_context(
            tc.tile_pool(name="poolps", bufs=1, space="PSUM"))

        ident = p["ident"].tile([128, 128], F32)
        make_identity(nc, ident[:])
        zrow = p["ident"].tile([1, 512], F32)
        nc.vector.memset(zrow[:], 0.0)
        zrowb = p["ident"].tile([1, 128], BF16)
        nc.vector.memset(zrowb[:], 0.0)
        if levels[1]["new_conv"]:
            nc.sync.dma_start(d["xt1"][0:1, :], zrowb[:1, :])
        else:
            nc.sync.dma_start(d["xt1"][0:1, :], zrow[:1, :levels[1]["xe"]])
        for i in range(2, 4):
            nc.sync.dma_start(d[f"xt{i}"][0:1, :], zrow[:1, :levels[i]["xe"]])
        nc.sync.dma_start(d["hta0"][0:1, :], zrow[:1, :128])
        for i in range(1, 4):
            nc.sync.dma_start(d[f"hta{i}"][0:1, :], zrow[:1, :levels[i]["he"]])

        for i, L in enumerate(levels):
            if L["new_conv"]:
                _emit_conv_new(nc, d, i, L, ident, p)
            else:
                _emit_conv_legacy(nc, d, i, L, ident, p)
            _emit_pool(nc, d, levels, i, L, ident, p)
        _emit_final(nc, d, p)


def _emit_conv_new(nc, d, i, L, ident, p):
    """Levels 0/1: transpose-gather bf16 conv (no PE assembly)."""
    C_in, M = L["C_in"], L["M"]
    Upad, sgc = L["Upad"], SGC[i]
    n_gi = Upad // sgc

    Wb = []
    for s in range(SEQ):
        Ks = L["Wbs"][s].shape[0]
        wt32 = p["w"].tile([Ks, M], F32, tag=f"Wb32_{i}_{s}")
        nc.sync.dma_start(wt32[:], d[f"Wb{i}_{s}"][:])
        wtb = p["w"].tile([Ks, M], BF16, tag=f"Wb_{i}_{s}")
        nc.vector.tensor_copy(wtb[:], wt32[:])
        Wb.append(wtb)

    for gi in range(n_gi):
        gts = []
        for s in range(SEQ):
            it = p["idx"].tile([128, sgc // 16], I16, tag="gidx")
            nc.sync.dma_start(
                it[:], d[f"gidx{i}"][:, (s * Upad + gi * sgc) // 16:
                                     (s * Upad + (gi + 1) * sgc) // 16])
            gt = p["g"].tile([128, sgc], BF16, tag=f"g{s}")
            src_ap = d[f"x0t{s}"][:] if i == 0 else d["xt1"][:]
            nc.gpsimd.dma_gather(
                gt[:].rearrange("p (o n) -> p o n", o=1),
                src_ap, it[:], sgc, sgc, 128,
                transpose=True, queue_num=next(p["qctr"]) % 4)
            gts.append(gt)
        for blk in range(sgc // 512):
            if blk % 2 == 0:
                cps = p["conv_ps"].tile([128, 512], F32, tag="conv")
            else:
                cps = p["asm_ps"].tile([128, 512], F32, tag="asm")
            for s in range(SEQ):
                Ks = L["Wbs"][s].shape[0]
                nc.tensor.matmul(
                    cps[:M, :],
                    lhsT=Wb[s][:Ks, :M],
                    rhs=gts[s][:Ks, blk * 512:(blk + 1) * 512],
                    start=(s == 0), stop=(s == SEQ - 1))
            # ELU: h = Relu(z) + exp(min(z, 0)) - 1
            tneg = p["tmp"].tile([128, 512], F32, tag="tneg")
            nc.vector.tensor_scalar_min(tneg[:M, :], cps[:M, :], 0.0)
            nc.scalar.activation(tneg[:M, :], tneg[:M, :], AF.Exp)
            hT = p["h"].tile([128, 512], F32, tag="hT")
            nc.scalar.activation(hT[:M, :], cps[:M, :], AF.Relu)
            nc.vector.tensor_add(hT[:M, :], hT[:M, :], tneg[:M, :])
            nc.vector.tensor_scalar_add(hT[:M, :], hT[:M, :], -1.0)
            # transpose 128-dest chunks and store h rows
            for cc in range(4):
                nps = p["nat_ps"].tile([128, 128], F32, tag="tp")
                nc.tensor.matmul(nps[:, :M],
                                 lhsT=hT[:M, cc * 128:(cc + 1) * 128],
                                 rhs=ident[:M, :M], is_transpose=True,
                                 start=True, stop=True)
                nsb = p["nat"].tile([128, 128], F32, tag="hnatsb")
                nc.vector.tensor_copy(nsb[:, :M], nps[:, :M])
                pos0 = gi * sgc + blk * 512 + cc * 128
                if i == 0:
                    t0 = pos0 // 2 + 1
                    nc.sync.dma_start(d["hta0"][t0: t0 + 64, 0:64],
                                      nsb[0:64, :64])
                    nc.sync.dma_start(d["hta0"][t0: t0 + 64, 64:128],
                                      nsb[64:128, :64])
                else:
                    nc.sync.dma_start(
                        d[f"hta{i}"][pos0 + 1: pos0 + 129, :M], nsb[:, :M])


def _emit_conv_legacy(nc, d, i, L, ident, p):
    C_in, C_out, M, n_mh = L["C_in"], L["C_out"], L["M"], L["n_mh"]
    xe = L["xe"]
    Upad = L["Upad"]
    sgc = SGC[i]
    kchunks = L["kchunks"]

    Wts = []
    for k, Wc in enumerate(L["Wcs"]):
        wt = p["w"].tile([Wc.shape[0], Wc.shape[1]], F32, tag=f"W{i}_{k}")
        nc.sync.dma_start(wt[:], d[f"W{i}_{k}"][:])
        Wts.append(wt)
    bias_t = p["w"].tile([128, n_mh], F32, tag=f"bias{i}")
    nc.sync.dma_start(bias_t[:], d[f"bias{i}"][:])

    n_gi = Upad // sgc
    nslab_g = sgc // 128
    packed = i == 0  # gapless 54-row chunk via DVE pack + one transpose
    sw = nslab_g * xe  # per-slot region width (1024 at every level)

    for gi in range(n_gi):
        gbig = p["g"].tile([128, SEQ * sw], F32, tag="gbig")
        gtiles = [gbig[:, s * sw:(s + 1) * sw] for s in range(SEQ)]
        for s in range(SEQ):
            it = p["idx"].tile([128, sgc // 16], I16, tag="gidx")
            nc.sync.dma_start(
                it[:], d[f"gidx{i}"][:, (s * Upad + gi * sgc) // 16:
                                     (s * Upad + (gi + 1) * sgc) // 16])
            src_ap = d[f"x0t{s}"][:] if i == 0 else d[f"xt{i}"][:]
            nc.gpsimd.dma_gather(
                gtiles[s].rearrange("p (n e) -> p n e", e=xe),
                src_ap, it[:], sgc, sgc, xe,
                single_packet=False, queue_num=next(p["qctr"]) % 4)
        if packed:
            kw2 = 2 * C_in           # 6
            Kp = SEQ * kw2           # 54
            g4 = gbig[:].rearrange("p (s w e) -> p s w e",
                                   s=SEQ, w=nslab_g)
            for blk in range(nslab_g // 4):
                cps = p["conv_ps"].tile([128, 512 * n_mh], F32, tag="conv")
                aps = p["asm_ps"].tile([128, 512], F32, tag="asm")
                for sub in range(4):
                    slab_i = blk * 4 + sub
                    pk = p["rhs"].tile([128, 64], F32, tag="pk")
                    nc.vector.tensor_copy(
                        pk[:, :Kp].rearrange("p (s o e) -> p s o e",
                                             s=SEQ, o=1),
                        g4[:, :, slab_i: slab_i + 1, 0: kw2])
                    nc.tensor.matmul(
                        aps[0: Kp, sub * 128:(sub + 1) * 128],
                        lhsT=pk[:, :Kp], rhs=ident[:],
                        is_transpose=True, start=True, stop=True)
                rb = p["rhs"].tile([128, 512], F32, tag="rhs")
                nc.vector.tensor_copy(rb[:Kp, :], aps[:Kp, :])
                nc.tensor.matmul(
                    cps[:M, :512], lhsT=Wts[0][:, :M],
                    rhs=rb[:Kp, :], start=True, stop=True)
                _emit_elu_store(nc, d, i, L, ident, p, bias_t, cps, gi, blk)
            continue
        for blk in range(nslab_g // 4):
            cps = p["conv_ps"].tile([128, 512 * n_mh], F32, tag="conv")
            for kci, ck in enumerate(kchunks):
                K = max(poff + kw for (_, _, kw, poff) in ck)
                aps = p["asm_ps"].tile([128, 512], F32, tag="asm")
                for sub in range(4):
                    slab_i = blk * 4 + sub
                    for (s, bsel, kw, poff) in ck:
                        c0 = slab_i * xe + (0 if bsel is None
                                            else bsel * C_in)
                        in_ap = gtiles[s][:, c0: c0 + kw]
                        nc.tensor.matmul(
                            aps[poff: poff + kw,
                                sub * 128:(sub + 1) * 128],
                            lhsT=in_ap, rhs=ident[:],
                            start=True, stop=True)
                rb = p["rhs"].tile([128, 512], F32, tag="rhs")
                nc.vector.tensor_copy(rb[:K, :], aps[:K, :])
                for mh in range(n_mh):
                    mw = min(128, M - mh * 128)
                    nc.tensor.matmul(
                        cps[:mw, mh * 512:(mh + 1) * 512],
                        lhsT=Wts[kci][:, mh * 128: mh * 128 + mw],
                        rhs=rb[:K, :], start=(kci == 0),
                        stop=(kci == len(kchunks) - 1))
            _emit_elu_store(nc, d, i, L, ident, p, bias_t, cps, gi, blk)


def _emit_elu_store(nc, d, i, L, ident, p, bias_t, cps, gi, blk):
    M, n_mh = L["M"], L["n_mh"]
    sgc = SGC[i]
    for mh in range(n_mh):
        mw = min(128, M - mh * 128)
        # bias + ELU: h = Relu(z+b) + exp(min(z+b, 0)) - 1
        hT = p["h"].tile([128, 512], F32, tag="hT")
        tneg = p["tmp"].tile([128, 512], F32, tag="tneg")
        bsl = bias_t[:mw, mh: mh + 1]
        csl = cps[:mw, mh * 512:(mh + 1) * 512]
        nc.vector.tensor_scalar(tneg[:mw, :], csl, bsl, 0.0,
                                mybir.AluOpType.add,
                                mybir.AluOpType.min)
        nc.scalar.activation(tneg[:mw, :], tneg[:mw, :], AF.Exp)
        nc.scalar.activation(hT[:mw, :], csl, AF.Relu, bias=bsl)
        nc.vector.tensor_add(hT[:mw, :], hT[:mw, :], tneg[:mw, :])
        nc.vector.tensor_scalar_add(hT[:mw, :], hT[:mw, :], -1.0)
        # naturalize 128-dest chunks and store h rows
        for cc in range(4):
            nps = p["nat_ps"].tile([128, 128], F32, tag="tp")
            nc.tensor.matmul(nps[:, :mw],
                             lhsT=hT[:mw, cc * 128:(cc + 1) * 128],
                             rhs=ident[:mw, :mw], is_transpose=True,
                             start=True, stop=True)
            nsb = p["nat"].tile([128, 128], F32, tag="hnatsb")
            nc.vector.tensor_copy(nsb[:, :mw], nps[:, :mw])
            pos0 = gi * sgc + blk * 512 + cc * 128
            if i == 0:
                t0 = pos0 // 2 + 1
                nc.sync.dma_start(d["hta0"][t0: t0 + 64, 0:64],
                                  nsb[0:64, :64])
                nc.sync.dma_start(d["hta0"][t0: t0 + 64, 64:128],
                                  nsb[64:128, :64])
            else:
                nc.sync.dma_start(
                    d[f"hta{i}"][pos0 + 1: pos0 + 129,
                                 mh * 128: mh * 128 + mw],
                    nsb[:, :mw])


def _emit_pool(nc, d, levels, i, L, ident, p):
    nslab, npad = L["nslab"], L["npad"]
    paired = L["paired"]
    n_half = 2 if paired else 1
    pgc = PGC[i]
    helem = 128 if paired else L["he"]
    M_next = 512 if i == 3 else (2 * levels[i + 1]["C_in"])
    n_mh_next = -(-M_next // 128)
    win_slabs = pgc // 128
    n_win = npad // pgc
    tiles = {}

    def issue(wi):
        it = p["idx"].tile([128, pgc // 16], I16, tag="pidx")
        nc.sync.dma_start(
            it[:], d[f"pidx{i}"][:, wi * pgc // 16: (wi + 1) * pgc // 16])
        gt = p["ph"].tile([128, win_slabs * helem], F32, tag="ph")
        nc.gpsimd.dma_gather(
            gt[:].rearrange("p (n e) -> p n e", e=helem),
            d[f"hta{i}"][:], it[:], pgc, pgc, helem,
            single_packet=False, queue_num=next(p["qctr"]) % 4)
        tiles[wi] = gt

    for wi in range(min(2, n_win)):
        issue(wi)

    xt1_bf = i == 0 and levels[1]["new_conv"]
    if xt1_bf:
        xsb_a = p["nat"].tile([128, 128], BF16, tag="xsb0")
        xsb_b = p["nat"].tile([128, 128], BF16, tag="xsb0")
        xsb = [xsb_a, xsb_b]
        for t in xsb:
            nc.vector.memset(t[:], 0.0)
            nc.vector.memset(t[:, 64:65], 1.0)

    for (g, s0, s1) in L["rblocks"]:
        n_rc = min(RBLOCK, L["N_out"] - g * RBLOCK)
        s2t = None
        if s1 > s0:
            s2t = p["s2"].tile([128, (s1 - s0) * n_half * RWIN], F32, tag="s2")
            nc.sync.dma_start(
                s2t[:], d[f"S2_{i}"][:, s0 * n_half * RWIN:
                                     s1 * n_half * RWIN])
        for mh in range(n_mh_next):
            mw = min(128, M_next - mh * 128)
            pps = p["pool_ps"].tile([128, RBLOCK], F32, tag="pool")
            nc.vector.memset(pps[:mw, :], 0.0)
            for si in range(s0, s1):
                wi, sub = divmod(si, win_slabs)
                if wi + 1 < n_win and wi + 1 not in tiles:
                    issue(wi + 1)
                (_, w_off) = L["slab_meta"][si]
                gt = tiles[wi]
                for hf in range(n_half):
                    nc.tensor.matmul(
                        pps[:mw, w_off: w_off + RWIN],
                        lhsT=gt[:, sub * helem + hf * 64 + mh * 128:
                                sub * helem + hf * 64 + mh * 128 + mw]
                        if paired else
                        gt[:, sub * helem + mh * 128:
                           sub * helem + mh * 128 + mw],
                        rhs=s2t[:, ((si - s0) * n_half + hf) * RWIN:
                                ((si - s0) * n_half + hf + 1) * RWIN],
                        start=False,
                        stop=(si == s1 - 1 and hf == n_half - 1),
                        skip_group_check=True)
            xTs = p["nat"].tile([128, RBLOCK], F32, tag="xT")
            nc.vector.tensor_copy(xTs[:mw, :], pps[:mw, :])
            if i == 3:
                # store x4^T plane directly in final-matmul layout
                # plane index p = 2h + b for mh = 2b + h
                b_, h_ = divmod(mh, 2)
                nc.sync.dma_start(d["x4p"][2 * h_ + b_],
                                  xTs[:mw, :256])
                continue
            for cc in range(-(-n_rc // 128)):
                ncc = min(128, n_rc - cc * 128)
                nps = p["nat_ps"].tile([128, 128], F32, tag="tp")
                nc.tensor.matmul(nps[:ncc, :mw],
                                 lhsT=xTs[:mw, cc * 128: cc * 128 + ncc],
                                 rhs=ident[:mw, :mw], is_transpose=True,
                                 start=True, stop=True)
                row0 = g * RBLOCK + cc * 128 + 1
                if xt1_bf:
                    nsb = xsb[cc % 2]
                    nc.vector.tensor_copy(nsb[:ncc, :64], nps[:ncc, :64])
                    nc.sync.dma_start(
                        d["xt1"][row0: row0 + ncc, :], nsb[:ncc, :])
                else:
                    nsb = p["nat"].tile([128, 128], F32, tag="xnatsb")
                    nc.vector.tensor_copy(nsb[:ncc, :mw], nps[:ncc, :mw])
                    nc.sync.dma_start(
                        d[f"xt{i + 1}"][row0: row0 + ncc,
                                        mh * 128: mh * 128 + mw],
                        nsb[:ncc, :mw])


def _emit_final(nc, d, p):
    # out[b, :] = x4flat[b] @ Wf + bf
    # lhsT = x4T K-chunk [128, B_LOC] (from x4p planes), rhs = Wf K-chunk
    # [128, LATENT]: wide-N accumulating matmuls (N=2 back-to-back
    # accumulation into the same PSUM columns loses updates on HW).
    fps = p["pool_ps"].tile([B_LOC, LATENT], F32, tag="pool")
    xt = p["fin"].tile([128, 4 * 256], F32, tag="x4T")
    for pl in range(4):
        nc.sync.dma_start(xt[:, pl * 256:(pl + 1) * 256], d["x4p"][pl])
    xtb = p["fin"].tile([128, 4 * 256], BF16, tag="x4Tb")
    nc.vector.tensor_copy(xtb[:], xt[:])
    bias_t = p["fin"].tile([B_LOC, LATENT], F32, tag="bf")
    for b in range(B_LOC):
        nc.sync.dma_start(bias_t[b: b + 1, :], d["bfv"][:].rearrange("l o -> o l"))
    xtb4 = xtb[:].rearrange("c (hb v) -> c v hb", hb=4)
    n_k = VERTS[4] * CH[4] // 128  # 512 K-chunks; 4 per Wf DMA
    for q in range(n_k // 4):
        wt = p["wfq"].tile([128, 4 * LATENT], BF16, tag="wfq")
        nc.sync.dma_start(
            wt[:].rearrange("p (f l) -> p f l", f=4),
            d["Wfb"][q * 512:(q + 1) * 512, :]
            .rearrange("(f p) l -> p f l", p=128))
        for f in range(4):
            kc = q * 4 + f
            v, h = divmod(kc, 2)
            nc.tensor.matmul(
                fps[:, :],
                lhsT=xtb4[:, v, 2 * h: 2 * h + 2],
                rhs=wt[:, f * LATENT:(f + 1) * LATENT],
                start=(kc == 0), stop=(kc == n_k - 1))
    osb = p["fin"].tile([B_LOC, LATENT], F32, tag="osb")
    nc.vector.tensor_add(osb[:], fps[:], bias_t[:])
    nc.sync.dma_start(d["out"][:], osb[:])


def kernel(**inputs) -> np.ndarray:
    levels = _host_prep(inputs)
    nc = _build_bass(levels)
    in_maps = [_build_in_map(levels, inputs, c) for c in range(N_CORES)]
    res = run_bass_kernel_spmd(nc, in_maps, core_ids=list(range(N_CORES)))
    return np.concatenate([res.results[c]["out"] for c in range(N_CORES)],
                          axis=0).astype(np.float32)


if __name__ == "__main__":
    sys.path.insert(0, "/root/problem")
    import reference
    inp = {k: np.asarray(v) for k, v in reference.setup_inputs().items()}
    got = kernel(**inp)
    exp = np.asarray(reference.reference(**inp))
    print("rel err:", np.abs(got - exp).max() / np.abs(exp).max())


# revision 27
# speedup vs baseline: 1.4997x; 1.0651x over previous
"""GNN spiral-conv encoder on 8 TRN2 NeuronCores (Bass/Tile).

Sharding: data-parallel over batch (2 of 16 samples per core); all index
structures replicated. Final Linear uses bf16 Wf replicated per core.

Per level: per-slot f32 x tables in HBM; dma_gather pulls, per used
destination vertex and spiral slot, the source row into [128 dest, elem]
slabs; PE transpose-matmuls assemble the conv rhs in PSUM; matmuls vs
host-built block-diagonal weight chunks accumulate; bias+ELU
(= Relu(z+b) + exp(min(z+b,0)) - 1) splits across DVE and the otherwise
idle ACT engine. h rows are transposed back (PE) into f32 HBM row tables.
(A transpose=True dma_gather path that would skip the PE assembly is
gated off: it crashes this HW/ucode build; see the new_conv flag.)

Level 0's h table packs two h rows per 512B table row (chunk-half pairing:
dest j and j+64 of each 128-chunk share a row) so pool-0 indices fit int16
in one pass; the pool then runs two masked matmuls per slab (half A/B
banded value matrices).

Pools: entries sorted by dest; window gathers are issued one ahead
(software pipeline, bufs=3) instead of on demand; PE matmul vs banded
value matrix accumulates x_{i+1}^T in PSUM; levels 1-3 keep the legacy
f32 tables and (for 2/3) the legacy assembly conv. Level-3 pool stores
the x4^T planes directly in the final matmul's layout (no transpose).
"""
import sys

sys.path.insert(0, "/opt/trn_rl_repo")

import numpy as np

import concourse.bass as bass
import concourse.tile as tile
from concourse import bacc, mybir
from concourse.bass_utils import run_bass_kernel_spmd
from concourse.library_config import mlp as _mlp_lib
from concourse.masks import make_identity

F32 = mybir.dt.float32
BF16 = mybir.dt.bfloat16
I16 = mybir.dt.int16
AF = mybir.ActivationFunctionType

VERTS = [65536, 16384, 4096, 1024, 256]
SEQ = 9
CH = [3, 32, 64, 128, 256]
LATENT = 256
B = 16
N_CORES = 8
B_LOC = B // N_CORES

SGC = [2048, 2048, 1024, 512]   # spiral gather idxs per instruction
PGC = [1024, 1024, 1024, 256]    # pool gather idxs per instruction
RWIN = 64                        # pool value-matrix r-window
RBLOCK = 512                     # pool PSUM r-block
HALF = 32768


def _wrap_idx16(idx, chunk):
    """[128, n/16] int16 dma_gather layout: within each `chunk` window,
    index i -> partition i%16, col i//16; replicated to all 8 groups."""
    idx = np.asarray(idx, dtype=np.int64)
    n = idx.shape[0]
    assert n % chunk == 0 and idx.max() < HALF and idx.min() >= 0
    nin = n // chunk
    w = idx.reshape(nin, chunk // 16, 16).astype(np.int16)
    blocks = [w[j].T for j in range(nin)]
    one = np.concatenate(blocks, axis=1)  # [16, n/16]
    return np.tile(one, (8, 1))           # [128, n/16]


def _pad_to(a, n, fill=0):
    out = np.full((n,) + a.shape[1:], fill, dtype=a.dtype)
    out[: a.shape[0]] = a
    return out


def _h0_row_half(hrow):
    """Level-0 paired h-table position for h row index (1-based; 0 = zero
    row). Dest pos p = hrow-1; chunk base cb = (p//128)*128; j = p%128;
    row = cb//2 + (j%64) + 1, half = j//64."""
    if hrow == 0:
        return 0, 0
    p = hrow - 1
    cb, j = divmod(p, 128)
    return cb * 64 + (j % 64) + 1, j // 64


def _build_level_host(i, idx, col, row, val, W, b):
    N_in, N_out = VERTS[i], VERTS[i + 1]
    C_in, C_out = CH[i], CH[i + 1]
    L = dict(N_in=N_in, N_out=N_out, C_in=C_in, C_out=C_out)
    L["xe"] = max(64, 2 * C_in)      # legacy x-table row width (f32)
    L["he"] = max(64, 2 * C_out)     # h-table row width (f32)
    M = 2 * C_out
    L["M"] = M
    L["n_mh"] = -(-M // 128)
    L["new_conv"] = False  # transpose-gather conv pending HW validation

    # used conv outputs (h rows) = unique pool source columns
    used = np.unique(col)
    U = used.shape[0]
    Upad = -(-U // SGC[i]) * SGC[i]
    L.update(used=used, U=U, Upad=Upad)

    # spiral gather index lists per slot s
    src = idx[used, :]  # [U, SEQ]
    gcols = []
    ref_lists = []
    for s in range(SEQ):
        ss = src[:, s]
        if i == 0:
            ref = np.unique(ss)
            assert ref.shape[0] < HALF, ref.shape
            loc = np.searchsorted(ref, ss) + 1
            ref_lists.append(ref)
        else:
            loc = ss + 1
        gcols.append(_wrap_idx16(_pad_to(loc.astype(np.int64), Upad), SGC[i]))
    L["gidx"] = np.concatenate(gcols, axis=1)  # [128, SEQ*Upad/16]
    L["ref_lists"] = ref_lists

    if L["new_conv"]:
        # per-slot block-diagonal weight chunks [Ks, M] (slot 0 gets a
        # bias row fed by the ones-component of its x table)
        Wbs = []
        for s in range(SEQ):
            Ks = 2 * C_in + (1 if s == 0 else 0)
            Wc = np.zeros((Ks, M), dtype=np.float32)
            for bb in range(B_LOC):
                Wc[bb * C_in:(bb + 1) * C_in,
                   bb * C_out:(bb + 1) * C_out] = W[s * C_in:(s + 1) * C_in, :]
                if s == 0:
                    Wc[2 * C_in, bb * C_out:(bb + 1) * C_out] = b
            Wbs.append(Wc)
        L["Wbs"] = Wbs
    else:
        # legacy conv K-chunks; level 0 instead packs all 9 slots into
        # one gapless 54-row chunk (strided DVE pack-copy + one PE
        # transpose per 128-dest sub-block)
        if i == 0:
            kw = 2 * C_in
            Wc = np.zeros((SEQ * kw, M), dtype=np.float32)
            for s in range(SEQ):
                for bb in range(B_LOC):
                    Wc[s * kw + bb * C_in: s * kw + (bb + 1) * C_in,
                       bb * C_out:(bb + 1) * C_out] = \
                        W[s * C_in:(s + 1) * C_in, :]
            L.update(kchunks=None, Wcs=[Wc])
        else:
            if 2 * C_in <= 128:
                kwidth = 2 * C_in
                raw = [(s, None) for s in range(SEQ)]
            else:
                kwidth = C_in
                raw = [(s, bb) for s in range(SEQ) for bb in range(B_LOC)]
            if kwidth <= 32:
                offs = [0, 32, 64]
            elif kwidth <= 64:
                offs = [0, 64]
            else:
                offs = [0]
            cs = len(offs)
            kchunks = []
            for j in range(0, len(raw), cs):
                kchunks.append([(s, bsel, kwidth, offs[t])
                                for t, (s, bsel) in enumerate(raw[j: j + cs])])
            Wcs = []
            for ck in kchunks:
                K = max(poff + kwidth for (_, _, _, poff) in ck)
                Wc = np.zeros((K, M), dtype=np.float32)
                for (s, bsel, kw, poff) in ck:
                    if bsel is None:
                        for bb in range(B_LOC):
                            Wc[poff + bb * C_in: poff + (bb + 1) * C_in,
                               bb * C_out:(bb + 1) * C_out] = \
                                W[s * C_in:(s + 1) * C_in, :]
                    else:
                        Wc[poff: poff + C_in,
                           bsel * C_out:(bsel + 1) * C_out] = \
                            W[s * C_in:(s + 1) * C_in, :]
                Wcs.append(Wc)
            L.update(kchunks=kchunks, Wcs=Wcs)
        bias_flat = np.repeat(b[None, :], B_LOC, axis=0).reshape(-1)  # [M]
        L["bias"] = _pad_to(bias_flat.astype(np.float32), L["n_mh"] * 128) \
            .reshape(L["n_mh"], 128).T.copy()  # [128, n_mh]

    # pool: sort entries by dest row, pack into slabs
    colpos = np.searchsorted(used, col)  # h row index - 1
    order = np.argsort(row, kind="stable")
    er, ec, ev = row[order], colpos[order] + 1, val[order]
    slabs, cur = [], []
    for k in range(er.shape[0]):
        r = int(er[k])
        if cur and (len(cur) >= 128 or r - cur[0][2] >= RWIN
                    or (r // RBLOCK) != (cur[0][2] // RBLOCK)):
            slabs.append(cur)
            cur = []
        cur.append((int(ec[k]), float(ev[k]), r))
    if cur:
        slabs.append(cur)
    nslab = len(slabs)
    paired = i == 0
    n_half = 2 if paired else 1
    pool_idx = np.zeros((nslab * 128,), dtype=np.int64)
    S2 = np.zeros((128, nslab * n_half * RWIN), dtype=np.float32)
    slab_meta = []
    for si, sl in enumerate(slabs):
        r0 = sl[0][2]
        g = r0 // RBLOCK
        w_off = r0 - g * RBLOCK
        if w_off + RWIN > RBLOCK:
            w_off = RBLOCK - RWIN
        slab_meta.append((g, w_off))
        for j, (hrow, v, r) in enumerate(sl):
            wpos = r - g * RBLOCK - w_off
            if paired:
                trow, half = _h0_row_half(hrow)
                pool_idx[si * 128 + j] = trow
                S2[j, (2 * si + half) * RWIN + wpos] = v
            else:
                pool_idx[si * 128 + j] = hrow
                S2[j, si * RWIN + wpos] = v
    rblocks = []
    cur_g, start = None, 0
    for si, (g, _) in enumerate(slab_meta):
        if g != cur_g:
            if cur_g is not None:
                rblocks.append((cur_g, start, si))
            cur_g, start = g, si
    if cur_g is not None:
        rblocks.append((cur_g, start, nslab))
    covered = {g for (g, _, _) in rblocks}
    for g in range(-(-N_out // RBLOCK)):
        if g not in covered:
            rblocks.append((g, 0, 0))
    rblocks.sort()
    npad = -(-nslab * 128 // PGC[i]) * PGC[i]
    L["pidx"] = _wrap_idx16(_pad_to(pool_idx, npad), PGC[i])
    L.update(nslab=nslab, paired=paired, S2=S2, slab_meta=slab_meta,
             rblocks=rblocks, npad=npad)
    return L


def _host_prep(inputs):
    g = lambda k: np.asarray(inputs[k])
    return [
        _build_level_host(i, g(f"idx{i}").astype(np.int64),
                          g(f"col{i}").astype(np.int64),
                          g(f"row{i}").astype(np.int64),
                          g(f"val{i}").astype(np.float32),
                          g(f"W{i}").astype(np.float32),
                          g(f"b{i}").astype(np.float32))
        for i in range(4)
    ]


def _build_in_map(levels, inputs, core):
    import ml_dtypes
    x = np.asarray(inputs["x"], dtype=np.float32)
    xs = x[core * B_LOC:(core + 1) * B_LOC]  # [2, N0, 3]
    m = {}
    L0 = levels[0]
    for s in range(SEQ):
        ref = L0["ref_lists"][s]
        if L0["new_conv"]:
            t = np.zeros((HALF, 128), dtype=ml_dtypes.bfloat16)
            for bb in range(B_LOC):
                t[1:1 + ref.shape[0], bb * CH[0]:(bb + 1) * CH[0]] = xs[bb][ref, :]
            if s == 0:
                t[1:1 + ref.shape[0], 2 * CH[0]] = 1.0  # bias ones-component
        else:
            t = np.zeros((HALF, 64), dtype=np.float32)
            for bb in range(B_LOC):
                t[1:1 + ref.shape[0], bb * CH[0]:(bb + 1) * CH[0]] = xs[bb][ref, :]
        m[f"x0t{s}"] = t
    for i, L in enumerate(levels):
        m[f"gidx{i}"] = L["gidx"]
        m[f"pidx{i}"] = L["pidx"]
        m[f"S2_{i}"] = L["S2"]
        if L["new_conv"]:
            for s, Wc in enumerate(L["Wbs"]):
                m[f"Wb{i}_{s}"] = Wc
        else:
            for k, Wc in enumerate(L["Wcs"]):
                m[f"W{i}_{k}"] = Wc
            m[f"bias{i}"] = L["bias"]
    m["Wfb"] = np.asarray(inputs["Wf"], dtype=np.float32).astype(ml_dtypes.bfloat16)
    m["bfv"] = np.asarray(inputs["bf"], dtype=np.float32)[:, None]
    return m


def _build_bass(levels):
    nc = bacc.Bacc("TRN2", target_bir_lowering=False, debug=False,
                   num_devices=N_CORES, num_swdge_queues=4)
    d = {}
    x0w, x0dt = ((128, BF16) if levels[0]["new_conv"] else (64, F32))
    for s in range(SEQ):
        d[f"x0t{s}"] = nc.dram_tensor(f"x0t{s}", [HALF, x0w], x0dt,
                                      kind="ExternalInput")
    for i, L in enumerate(levels):
        d[f"gidx{i}"] = nc.dram_tensor(
            f"gidx{i}", [128, SEQ * L["Upad"] // 16], I16, kind="ExternalInput")
        d[f"pidx{i}"] = nc.dram_tensor(
            f"pidx{i}", [128, L["npad"] // 16], I16, kind="ExternalInput")
        n_half = 2 if L["paired"] else 1
        d[f"S2_{i}"] = nc.dram_tensor(
            f"S2_{i}", [128, L["nslab"] * n_half * RWIN], F32,
            kind="ExternalInput")
        if L["new_conv"]:
            for s, Wc in enumerate(L["Wbs"]):
                d[f"Wb{i}_{s}"] = nc.dram_tensor(
                    f"Wb{i}_{s}", list(Wc.shape), F32, kind="ExternalInput")
        else:
            for k, Wc in enumerate(L["Wcs"]):
                d[f"W{i}_{k}"] = nc.dram_tensor(f"W{i}_{k}", list(Wc.shape),
                                                F32, kind="ExternalInput")
            d[f"bias{i}"] = nc.dram_tensor(f"bias{i}", [128, L["n_mh"]], F32,
                                           kind="ExternalInput")
        if i == 1:
            if L["new_conv"]:
                # bf16 x table, ones-component at col 2*C_in for the bias row
                d["xt1"] = nc.dram_tensor("xt1", [L["N_in"] + 1, 128], BF16,
                                          kind="Internal")
            else:
                d["xt1"] = nc.dram_tensor("xt1", [L["N_in"] + 1, L["xe"]],
                                          F32, kind="Internal")
        elif i > 1:
            d[f"xt{i}"] = nc.dram_tensor(f"xt{i}", [L["N_in"] + 1, L["xe"]],
                                         F32, kind="Internal")
        if i == 0:
            d["hta0"] = nc.dram_tensor("hta0", [L["Upad"] // 2 + 1, 128], F32,
                                       kind="Internal")
        else:
            d[f"hta{i}"] = nc.dram_tensor(f"hta{i}", [L["Upad"] + 1, L["he"]],
                                          F32, kind="Internal")
    d["x4p"] = nc.dram_tensor("x4p", [4, 128, 256], F32, kind="Internal")
    d["Wfb"] = nc.dram_tensor("Wfb", [VERTS[4] * CH[4], LATENT], BF16,
                              kind="ExternalInput")
    d["bfv"] = nc.dram_tensor("bfv", [LATENT, 1], F32, kind="ExternalInput")
    d["out"] = nc.dram_tensor("out", [B_LOC, LATENT], F32, kind="ExternalOutput")

    with tile.TileContext(nc) as tc:
        nc.gpsimd.load_library(_mlp_lib)
        _emit(nc, tc, d, levels)
    nc.compile()
    return nc


def _emit(nc, tc, d, levels):
    from contextlib import ExitStack
    import itertools
    with ExitStack() as ctx:
        p = {}
        # SWDGE sem lanes are assigned round-robin over Pool-engine DMA
        # instructions in emission order (8 lanes); queue_num must follow
        # emission order so each lane stays locked to one queue.
        p["qctr"] = itertools.count()
        p["ident"] = ctx.enter_context(tc.tile_pool(name="ident", bufs=1))
        p["w"] = ctx.enter_context(tc.tile_pool(name="wp", bufs=1))
        p["idx"] = ctx.enter_context(tc.tile_pool(name="idxp", bufs=9))
        p["g"] = ctx.enter_context(tc.tile_pool(name="gp", bufs=2))
        p["rhs"] = ctx.enter_context(tc.tile_pool(name="rhsp", bufs=3))
        p["h"] = ctx.enter_context(tc.tile_pool(name="hp", bufs=3))
        p["tmp"] = ctx.enter_context(tc.tile_pool(name="tmpp", bufs=2))
        p["nat"] = ctx.enter_context(tc.tile_pool(name="natp", bufs=2))
        p["s2"] = ctx.enter_context(tc.tile_pool(name="s2p", bufs=2))
        p["ph"] = ctx.enter_context(tc.tile_pool(name="php", bufs=4))
        p["fin"] = ctx.enter_context(tc.tile_pool(name="finp", bufs=1))
        p["wfq"] = ctx.enter_context(tc.tile_pool(name="wfqp", bufs=3))
        p["asm_ps"] = ctx.enter_context(
            tc.tile_pool(name="asmps", bufs=1, space="PSUM"))
        p["conv_ps"] = ctx.enter_context(
            tc.tile_pool(name="convps", bufs=1, space="PSUM"))
        p["nat_ps"] = ctx.enter_context(
            tc.tile_pool(name="natps", bufs=2, space="PSUM"))
        p["pool_ps"] = ctx.enter_context(
            tc.tile_pool(name="poolps", bufs=1, space="PSUM"))

        ident = p["ident"].tile([128, 128], F32)
        make_identity(nc, ident[:])
        zrow = p["ident"].tile([1, 512], F32)
        nc.vector.memset(zrow[:], 0.0)
        zrowb = p["ident"].tile([1, 128], BF16)
        nc.vector.memset(zrowb[:], 0.0)
        if levels[1]["new_conv"]:
            nc.sync.dma_start(d["xt1"][0:1, :], zrowb[:1, :])
        else:
            nc.sync.dma_start(d["xt1"][0:1, :], zrow[:1, :levels[1]["xe"]])
        for i in range(2, 4):
            nc.sync.dma_start(d[f"xt{i}"][0:1, :], zrow[:1, :levels[i]["xe"]])
        nc.sync.dma_start(d["hta0"][0:1, :], zrow[:1, :128])
        for i in range(1, 4):
            nc.sync.dma_start(d[f"hta{i}"][0:1, :], zrow[:1, :levels[i]["he"]])

        for i, L in enumerate(levels):
            if L["new_conv"]:
                _emit_conv_new(nc, d, i, L, ident, p)
            else:
                _emit_conv_legacy(nc, d, i, L, ident, p)
            _emit_pool(nc, d, levels, i, L, ident, p)
        _emit_final(nc, d, p)


def _emit_conv_new(nc, d, i, L, ident, p):
    """Levels 0/1: transpose-gather bf16 conv (no PE assembly)."""
    C_in, M = L["C_in"], L["M"]
    Upad, sgc = L["Upad"], SGC[i]
    n_gi = Upad // sgc

    Wb = []
    for s in range(SEQ):
        Ks = L["Wbs"][s].shape[0]
        wt32 = p["w"].tile([Ks, M], F32, tag=f"Wb32_{i}_{s}")
        nc.sync.dma_start(wt32[:], d[f"Wb{i}_{s}"][:])
        wtb = p["w"].tile([Ks, M], BF16, tag=f"Wb_{i}_{s}")
        nc.vector.tensor_copy(wtb[:], wt32[:])
        Wb.append(wtb)

    for gi in range(n_gi):
        gts = []
        for s in range(SEQ):
            it = p["idx"].tile([128, sgc // 16], I16, tag="gidx")
            nc.sync.dma_start(
                it[:], d[f"gidx{i}"][:, (s * Upad + gi * sgc) // 16:
                                     (s * Upad + (gi + 1) * sgc) // 16])
            gt = p["g"].tile([128, sgc], BF16, tag=f"g{s}")
            src_ap = d[f"x0t{s}"][:] if i == 0 else d["xt1"][:]
            nc.gpsimd.dma_gather(
                gt[:].rearrange("p (o n) -> p o n", o=1),
                src_ap, it[:], sgc, sgc, 128,
                transpose=True, queue_num=next(p["qctr"]) % 4)
            gts.append(gt)
        for blk in range(sgc // 512):
            if blk % 2 == 0:
                cps = p["conv_ps"].tile([128, 512], F32, tag="conv")
            else:
                cps = p["asm_ps"].tile([128, 512], F32, tag="asm")
            for s in range(SEQ):
                Ks = L["Wbs"][s].shape[0]
                nc.tensor.matmul(
                    cps[:M, :],
                    lhsT=Wb[s][:Ks, :M],
                    rhs=gts[s][:Ks, blk * 512:(blk + 1) * 512],
                    start=(s == 0), stop=(s == SEQ - 1))
            # ELU: h = Relu(z) + exp(min(z, 0)) - 1
            tneg = p["tmp"].tile([128, 512], F32, tag="tneg")
            nc.vector.tensor_scalar_min(tneg[:M, :], cps[:M, :], 0.0)
            nc.scalar.activation(tneg[:M, :], tneg[:M, :], AF.Exp)
            hT = p["h"].tile([128, 512], F32, tag="hT")
            nc.scalar.activation(hT[:M, :], cps[:M, :], AF.Relu)
            nc.vector.tensor_add(hT[:M, :], hT[:M, :], tneg[:M, :])
            nc.vector.tensor_scalar_add(hT[:M, :], hT[:M, :], -1.0)
            # transpose 128-dest chunks and store h rows
            for cc in range(4):
                nps = p["nat_ps"].tile([128, 128], F32, tag="tp")
                nc.tensor.matmul(nps[:, :M],
                                 lhsT=hT[:M, cc * 128:(cc + 1) * 128],
                                 rhs=ident[:M, :M], is_transpose=True,
                                 start=True, stop=True)
                nsb = p["nat"].tile([128, 128], F32, tag="hnatsb")
                nc.vector.tensor_copy(nsb[:, :M], nps[:, :M])
                pos0 = gi * sgc + blk * 512 + cc * 128
                if i == 0:
                    t0 = pos0 // 2 + 1
                    nc.sync.dma_start(d["hta0"][t0: t0 + 64, 0:64],
                                      nsb[0:64, :64])
                    nc.sync.dma_start(d["hta0"][t0: t0 + 64, 64:128],
                                      nsb[64:128, :64])
                else:
                    nc.sync.dma_start(
                        d[f"hta{i}"][pos0 + 1: pos0 + 129, :M], nsb[:, :M])


def _emit_conv_legacy(nc, d, i, L, ident, p):
    C_in, C_out, M, n_mh = L["C_in"], L["C_out"], L["M"], L["n_mh"]
    xe = L["xe"]
    Upad = L["Upad"]
    sgc = SGC[i]
    kchunks = L["kchunks"]

    Wts = []
    for k, Wc in enumerate(L["Wcs"]):
        wt = p["w"].tile([Wc.shape[0], Wc.shape[1]], F32, tag=f"W{i}_{k}")
        nc.sync.dma_start(wt[:], d[f"W{i}_{k}"][:])
        Wts.append(wt)
    bias_t = p["w"].tile([128, n_mh], F32, tag=f"bias{i}")
    nc.sync.dma_start(bias_t[:], d[f"bias{i}"][:])

    n_gi = Upad // sgc
    nslab_g = sgc // 128
    packed = i == 0  # gapless 54-row chunk via DVE pack + one transpose
    sw = nslab_g * xe  # per-slot region width (1024 at every level)

    for gi in range(n_gi):
        gbig = p["g"].tile([128, SEQ * sw], F32, tag="gbig")
        gtiles = [gbig[:, s * sw:(s + 1) * sw] for s in range(SEQ)]
        for s in range(SEQ):
            it = p["idx"].tile([128, sgc // 16], I16, tag="gidx")
            nc.sync.dma_start(
                it[:], d[f"gidx{i}"][:, (s * Upad + gi * sgc) // 16:
                                     (s * Upad + (gi + 1) * sgc) // 16])
            src_ap = d[f"x0t{s}"][:] if i == 0 else d[f"xt{i}"][:]
            nc.gpsimd.dma_gather(
                gtiles[s].rearrange("p (n e) -> p n e", e=xe),
                src_ap, it[:], sgc, sgc, xe,
                single_packet=False, queue_num=next(p["qctr"]) % 4)
        if packed:
            kw2 = 2 * C_in           # 6
            Kp = SEQ * kw2           # 54
            g4 = gbig[:].rearrange("p (s w e) -> p s w e",
                                   s=SEQ, w=nslab_g)
            for blk in range(nslab_g // 4):
                cps = p["conv_ps"].tile([128, 512 * n_mh], F32, tag="conv")
                aps = p["asm_ps"].tile([128, 512], F32, tag="asm")
                for sub in range(4):
                    slab_i = blk * 4 + sub
                    pk = p["rhs"].tile([128, 64], F32, tag="pk")
                    nc.vector.tensor_copy(
                        pk[:, :Kp].rearrange("p (s o e) -> p s o e",
                                             s=SEQ, o=1),
                        g4[:, :, slab_i: slab_i + 1, 0: kw2])
                    nc.tensor.matmul(
                        aps[0: Kp, sub * 128:(sub + 1) * 128],
                        lhsT=pk[:, :Kp], rhs=ident[:],
                        is_transpose=True, start=True, stop=True)
                rb = p["rhs"].tile([128, 512], F32, tag="rhs")
                nc.vector.tensor_copy(rb[:Kp, :], aps[:Kp, :])
                nc.tensor.matmul(
                    cps[:M, :512], lhsT=Wts[0][:, :M],
                    rhs=rb[:Kp, :], start=True, stop=True)
                _emit_elu_store(nc, d, i, L, ident, p, bias_t, cps, gi, blk)
            continue
        for blk in range(nslab_g // 4):
            cps = p["conv_ps"].tile([128, 512 * n_mh], F32, tag="conv")
            for kci, ck in enumerate(kchunks):
                K = max(poff + kw for (_, _, kw, poff) in ck)
                aps = p["asm_ps"].tile([128, 512], F32, tag="asm")
                for sub in range(4):
                    slab_i = blk * 4 + sub
                    for (s, bsel, kw, poff) in ck:
                        c0 = slab_i * xe + (0 if bsel is None
                                            else bsel * C_in)
                        in_ap = gtiles[s][:, c0: c0 + kw]
                        nc.tensor.matmul(
                            aps[poff: poff + kw,
                                sub * 128:(sub + 1) * 128],
                            lhsT=in_ap, rhs=ident[:],
                            start=True, stop=True)
                rb = p["rhs"].tile([128, 512], F32, tag="rhs")
                nc.vector.tensor_copy(rb[:K, :], aps[:K, :])
                for mh in range(n_mh):
                    mw = min(128, M - mh * 128)
                    nc.tensor.matmul(
                        cps[:mw, mh * 512:(mh + 1) * 512],
                        lhsT=Wts[kci][:, mh * 128: mh * 128 + mw],
                        rhs=rb[:K, :], start=(kci == 0),
                        stop=(kci == len(kchunks) - 1))
            _emit_elu_store(nc, d, i, L, ident, p, bias_t, cps, gi, blk)


def _emit_elu_store(nc, d, i, L, ident, p, bias_t, cps, gi, blk):
    M, n_mh = L["M"], L["n_mh"]
    sgc = SGC[i]
    for mh in range(n_mh):
        mw = min(128, M - mh * 128)
        # bias + ELU: h = Relu(z+b) + exp(min(z+b, 0)) - 1
        hT = p["h"].tile([128, 512], F32, tag="hT")
        tneg = p["tmp"].tile([128, 512], F32, tag="tneg")
        bsl = bias_t[:mw, mh: mh + 1]
        csl = cps[:mw, mh * 512:(mh + 1) * 512]
        nc.vector.tensor_scalar(tneg[:mw, :], csl, bsl, 0.0,
                                mybir.AluOpType.add,
                                mybir.AluOpType.min)
        nc.scalar.activation(tneg[:mw, :], tneg[:mw, :], AF.Exp)
        nc.scalar.activation(hT[:mw, :], csl, AF.Relu, bias=bsl)
        nc.vector.tensor_add(hT[:mw, :], hT[:mw, :], tneg[:mw, :])
        nc.vector.tensor_scalar_add(hT[:mw, :], hT[:mw, :], -1.0)
        # naturalize 128-dest chunks and store h rows
        for cc in range(4):
            nps = p["nat_ps"].tile([128, 128], F32, tag="tp")
            nc.tensor.matmul(nps[:, :mw],
                             lhsT=hT[:mw, cc * 128:(cc + 1) * 128],
                             rhs=ident[:mw, :mw], is_transpose=True,
                             start=True, stop=True)
            nsb = p["nat"].tile([128, 128], F32, tag="hnatsb")
            nc.vector.tensor_copy(nsb[:, :mw], nps[:, :mw])
            pos0 = gi * sgc + blk * 512 + cc * 128
            if i == 0:
                t0 = pos0 // 2 + 1
                nc.scalar.dma_start(d["hta0"][t0: t0 + 64, 0:64],
                                    nsb[0:64, :64])
                nc.scalar.dma_start(d["hta0"][t0: t0 + 64, 64:128],
                                    nsb[64:128, :64])
            else:
                nc.scalar.dma_start(
                    d[f"hta{i}"][pos0 + 1: pos0 + 129,
                                 mh * 128: mh * 128 + mw],
                    nsb[:, :mw])


def _emit_pool(nc, d, levels, i, L, ident, p):
    nslab, npad = L["nslab"], L["npad"]
    paired = L["paired"]
    n_half = 2 if paired else 1
    pgc = PGC[i]
    helem = 128 if paired else L["he"]
    M_next = 512 if i == 3 else (2 * levels[i + 1]["C_in"])
    n_mh_next = -(-M_next // 128)
    win_slabs = pgc // 128
    n_win = npad // pgc
    tiles = {}

    def issue(wi):
        it = p["idx"].tile([128, pgc // 16], I16, tag="pidx")
        nc.sync.dma_start(
            it[:], d[f"pidx{i}"][:, wi * pgc // 16: (wi + 1) * pgc // 16])
        gt = p["ph"].tile([128, win_slabs * helem], F32, tag="ph")
        nc.gpsimd.dma_gather(
            gt[:].rearrange("p (n e) -> p n e", e=helem),
            d[f"hta{i}"][:], it[:], pgc, pgc, helem,
            single_packet=False, queue_num=next(p["qctr"]) % 4)
        tiles[wi] = gt

    for wi in range(min(2, n_win)):
        issue(wi)

    xt1_bf = i == 0 and levels[1]["new_conv"]
    if xt1_bf:
        xsb_a = p["nat"].tile([128, 128], BF16, tag="xsb0")
        xsb_b = p["nat"].tile([128, 128], BF16, tag="xsb0")
        xsb = [xsb_a, xsb_b]
        for t in xsb:
            nc.vector.memset(t[:], 0.0)
            nc.vector.memset(t[:, 64:65], 1.0)

    for (g, s0, s1) in L["rblocks"]:
        n_rc = min(RBLOCK, L["N_out"] - g * RBLOCK)
        s2t = None
        if s1 > s0:
            s2t = p["s2"].tile([128, (s1 - s0) * n_half * RWIN], F32, tag="s2")
            nc.sync.dma_start(
                s2t[:], d[f"S2_{i}"][:, s0 * n_half * RWIN:
                                     s1 * n_half * RWIN])
        for mh in range(n_mh_next):
            mw = min(128, M_next - mh * 128)
            pps = p["pool_ps"].tile([128, RBLOCK], F32, tag="pool")
            nc.vector.memset(pps[:mw, :], 0.0)
            for si in range(s0, s1):
                wi, sub = divmod(si, win_slabs)
                if wi + 1 < n_win and wi + 1 not in tiles:
                    issue(wi + 1)
                (_, w_off) = L["slab_meta"][si]
                gt = tiles[wi]
                for hf in range(n_half):
                    nc.tensor.matmul(
                        pps[:mw, w_off: w_off + RWIN],
                        lhsT=gt[:, sub * helem + hf * 64 + mh * 128:
                                sub * helem + hf * 64 + mh * 128 + mw]
                        if paired else
                        gt[:, sub * helem + mh * 128:
                           sub * helem + mh * 128 + mw],
                        rhs=s2t[:, ((si - s0) * n_half + hf) * RWIN:
                                ((si - s0) * n_half + hf + 1) * RWIN],
                        start=False,
                        stop=(si == s1 - 1 and hf == n_half - 1),
                        skip_group_check=True)
            xTs = p["nat"].tile([128, RBLOCK], F32, tag="xT")
            nc.vector.tensor_copy(xTs[:mw, :], pps[:mw, :])
            if i == 3:
                # store x4^T plane directly in final-matmul layout
                # plane index p = 2h + b for mh = 2b + h
                b_, h_ = divmod(mh, 2)
                nc.scalar.dma_start(d["x4p"][2 * h_ + b_],
                                    xTs[:mw, :256])
                continue
            for cc in range(-(-n_rc // 128)):
                ncc = min(128, n_rc - cc * 128)
                nps = p["nat_ps"].tile([128, 128], F32, tag="tp")
                nc.tensor.matmul(nps[:ncc, :mw],
                                 lhsT=xTs[:mw, cc * 128: cc * 128 + ncc],
                                 rhs=ident[:mw, :mw], is_transpose=True,
                                 start=True, stop=True)
                row0 = g * RBLOCK + cc * 128 + 1
                if xt1_bf:
                    nsb = xsb[cc % 2]
                    nc.vector.tensor_copy(nsb[:ncc, :64], nps[:ncc, :64])
                    nc.scalar.dma_start(
                        d["xt1"][row0: row0 + ncc, :], nsb[:ncc, :])
                else:
                    nsb = p["nat"].tile([128, 128], F32, tag="xnatsb")
                    nc.vector.tensor_copy(nsb[:ncc, :mw], nps[:ncc, :mw])
                    nc.scalar.dma_start(
                        d[f"xt{i + 1}"][row0: row0 + ncc,
                                        mh * 128: mh * 128 + mw],
                        nsb[:ncc, :mw])


def _emit_final(nc, d, p):
    # out[b, :] = x4flat[b] @ Wf + bf
    # lhsT = x4T K-chunk [128, B_LOC] (from x4p planes), rhs = Wf K-chunk
    # [128, LATENT]: wide-N accumulating matmuls (N=2 back-to-back
    # accumulation into the same PSUM columns loses updates on HW).
    fps = p["pool_ps"].tile([B_LOC, LATENT], F32, tag="pool")
    xt = p["fin"].tile([128, 4 * 256], F32, tag="x4T")
    for pl in range(4):
        nc.sync.dma_start(xt[:, pl * 256:(pl + 1) * 256], d["x4p"][pl])
    xtb = p["fin"].tile([128, 4 * 256], BF16, tag="x4Tb")
    nc.vector.tensor_copy(xtb[:], xt[:])
    bias_t = p["fin"].tile([B_LOC, LATENT], F32, tag="bf")
    for b in range(B_LOC):
        nc.sync.dma_start(bias_t[b: b + 1, :], d["bfv"][:].rearrange("l o -> o l"))
    xtb4 = xtb[:].rearrange("c (hb v) -> c v hb", hb=4)
    n_k = VERTS[4] * CH[4] // 128  # 512 K-chunks; 4 per Wf DMA
    for q in range(n_k // 4):
        wt = p["wfq"].tile([128, 4 * LATENT], BF16, tag="wfq")
        nc.sync.dma_start(
            wt[:].rearrange("p (f l) -> p f l", f=4),
            d["Wfb"][q * 512:(q + 1) * 512, :]
            .rearrange("(f p) l -> p f l", p=128))
        for f in range(4):
            kc = q * 4 + f
            v, h = divmod(kc, 2)
            nc.tensor.matmul(
                fps[:, :],
                lhsT=xtb4[:, v, 2 * h: 2 * h + 2],
                rhs=wt[:, f * LATENT:(f + 1) * LATENT],
                start=(kc == 0), stop=(kc == n_k - 1))
    osb = p["fin"].tile([B_LOC, LATENT], F32, tag="osb")
    nc.vector.tensor_add(osb[:], fps[:], bias_t[:])
    nc.sync.dma_start(d["out"][:], osb[:])


def kernel(**inputs) -> np.ndarray:
    levels = _host_prep(inputs)
    nc = _build_bass(levels)
    in_maps = [_build_in_map(levels, inputs, c) for c in range(N_CORES)]
    res = run_bass_kernel_spmd(nc, in_maps, core_ids=list(range(N_CORES)))
    return np.concatenate([res.results[c]["out"] for c in range(N_CORES)],
                          axis=0).astype(np.float32)


if __name__ == "__main__":
    sys.path.insert(0, "/root/problem")
    import reference
    inp = {k: np.asarray(v) for k, v in reference.setup_inputs().items()}
    got = kernel(**inp)
    exp = np.asarray(reference.reference(**inp))
    print("rel err:", np.abs(got - exp).max() / np.abs(exp).max())
